# revision 26
# baseline (speedup 1.0000x reference)
"""Trainium2 Bass kernel for nn_EquivSetConv (hypergraph message passing).

Reference computation:
    Xve = (X @ W1 + b1)[vertex]
    Xe  = segment_sum(Xve, edges, M)
    Xev = Xe[edges]
    H   = concat([X[vertex], Xev], -1) @ W2 + b2
    Xv  = segment_sum(H, vertex, N)
    out = ((1-a)*Xv + a*X0) @ W3 + b3

Algebraic restructure (A[v,e] = #incidence pairs (v,e)):
    Se  = A^T @ X                          (segmented sum of raw X rows per edge)
    Xe  = Se @ W1 + edeg x b1
    T   = A @ Se                           (segmented sum of Se rows per vertex)
    Xv  = deg . (X @ W2a) + T @ (W1 @ W2b) + deg x b2 + wdeg x (b1 @ W2b)
    out = ((1-a)Xv + a X0) @ W3 + b3

So the 800k-row dense matmul disappears; the kernel is two sparse
gather+segmented-sum stages plus small dense matmuls.

Sharding over 8 cores: stage A partitioned by edge range (each core owns
M/8 edges and all pairs incident to them -> computes its Se slice fully,
no cross-core reduction), one AllGather of Se, stage B partitioned by
vertex range (each core owns N/8 vertices -> computes its output rows
end to end). The only collective is the 1.6MB/rank AllGather.

Sparse stages on device: host sorts pairs by destination segment and
packs them into 128-pair chunks that are pure in a 128-wide segment
window.  For each chunk: dma_gather 128 source rows (pair p -> SBUF
partition p), build a one-hot [pair, segment-slot] matrix on DVE
(iota == seg), and accumulate with one PE matmul into the window's PSUM
tile.  Windows flush to SBUF/DRAM when complete.
"""

import numpy as np

P = 128
D = 128


# ---------------------------------------------------------------------------
# host-side preprocessing
# ---------------------------------------------------------------------------

def _pack_stream(seg_local, gidx, n_windows, chunk_counts):
    """Pack pairs (sorted by window) into window-pure 128-slot chunks.

    seg_local: [n] int, segment id LOCAL to the stream's window grid
               (seg_local // 128 = window, seg_local % 128 = slot)
    gidx:      [n] int, gather index of each pair
    chunk_counts: [n_windows] int, chunks allocated per window (shared
               across all cores so the program structure is identical).

    Returns (idx16, segf) flat arrays of length sum(chunk_counts)*128,
    pad slots have idx 0 / seg -1.
    """
    total_chunks = int(np.sum(chunk_counts))
    tot = total_chunks * P
    idx16 = np.zeros(tot, dtype=np.int16)
    segf = np.full(tot, -1.0, dtype=np.float32)
    if len(seg_local) == 0:
        return idx16, segf

    order = np.argsort(seg_local, kind="stable")
    seg_s = seg_local[order]
    gidx_s = gidx[order]
    win = seg_s // P

    # position of each pair: chunk_base[win]*128 + rank-within-window
    chunk_base = np.concatenate([[0], np.cumsum(chunk_counts)[:-1]])
    win_start = np.searchsorted(win, np.arange(n_windows), side="left")
    rank = np.arange(len(win)) - win_start[win]
    pos = chunk_base[win] * P + rank
    idx16[pos] = gidx_s.astype(np.int16)
    segf[pos] = (seg_s % P).astype(np.float32)
    return idx16, segf


def _wrap_idx(idx16, G):
    """Reshape a flat per-stream idx array into the dma_gather SBUF layout.

    Within each batch of G*128 indices, index i lives at
    [partition i%16, column i//16]; batches are side by side (the last
    batch may cover fewer than G chunks; it is zero-padded for layout).
    Output [128, ceil(total_chunks/G)*G*8] int16 (rows 0..15 replicated).
    """
    rem = (-len(idx16)) % (G * P)
    if rem:
        idx16 = np.concatenate([idx16, np.zeros(rem, dtype=np.int16)])
    nb = len(idx16) // (G * P)
    blocks = [idx16[b * G * P:(b + 1) * G * P].reshape(G * 8, 16).T for b in range(nb)]
    arr16 = np.hstack(blocks)  # [16, nb*G*8]
    return np.tile(arr16, (8, 1)).astype(np.int16)


def _seg_tile(segf):
    """[total_chunks*128] -> [128, total_chunks]: pair (chunk c, part p)."""
    import ml_dtypes
    n_chunks = len(segf) // P
    return np.ascontiguousarray(segf.reshape(n_chunks, P).T).astype(ml_dtypes.bfloat16)


def _chunk_counts(windows_per_core, n_windows, G, min_one=True):
    """windows_per_core: list over cores of [n_windows] pair counts.
    Returns per-window chunk counts (max over cores), padded so the
    total is a multiple of G."""
    counts = np.zeros(n_windows, dtype=np.int64)
    for wc in windows_per_core:
        counts = np.maximum(counts, (wc + P - 1) // P)
    if min_one:
        counts = np.maximum(counts, 1)
    tot = int(counts.sum())
    counts[-1] += (-tot) % G
    return counts


def _balance_bins(loads, n_bins, cap):
    """LPT greedy: assign items (with given loads) to n_bins bins of
    capacity `cap` items each, minimizing max bin load.
    Returns [len(loads)] -> new id (bin*cap + slot-within-bin)."""
    import heapq
    order = np.argsort(-loads, kind="stable")
    heap = [(0, b) for b in range(n_bins)]
    heapq.heapify(heap)
    fill = np.zeros(n_bins, dtype=np.int64)
    newid = np.empty(len(loads), dtype=np.int64)
    spill = []
    for it in order:
        load, b = heapq.heappop(heap)
        newid[it] = b * cap + fill[b]
        fill[b] += 1
        if fill[b] < cap:
            heapq.heappush(heap, (load + loads[it], b))
        else:
            spill.append((load + loads[it], b))
    return newid


def preprocess(X, vertex, edges, X0, W1, b1, W2, b2, W3, b3,
               M=25000, ncores=8, G=8, lo_split=32768):
    """Build per-core input maps + compile-time metadata."""
    X = np.asarray(X, dtype=np.float32)
    X0 = np.asarray(X0, dtype=np.float32)
    vertex = np.asarray(vertex).astype(np.int64)
    edges = np.asarray(edges).astype(np.int64)
    W1 = np.asarray(W1, dtype=np.float32)
    b1 = np.asarray(b1, dtype=np.float32)
    W2 = np.asarray(W2, dtype=np.float32)
    b2 = np.asarray(b2, dtype=np.float32)
    W3 = np.asarray(W3, dtype=np.float32)
    b3 = np.asarray(b3, dtype=np.float32)

    N, Din = X.shape
    NNZ = len(vertex)
    Dout = W3.shape[1]
    assert Din == D and Dout == D

    LO = min(lo_split, N)
    EPC = M // ncores
    VPC = N // ncores
    assert M % ncores == 0 and N % ncores == 0
    NW2 = (EPC + P - 1) // P
    NW3 = (VPC + P - 1) // P

    alpha = 0.5
    W2a = W2[:D]
    W2b = W2[D:]
    deg = np.bincount(vertex, minlength=N).astype(np.float64)
    edeg = np.bincount(edges, minlength=M).astype(np.float64)
    wdeg = np.bincount(vertex, weights=edeg[edges], minlength=N)

    Wa = ((1.0 - alpha) * W2a).astype(np.float32)
    Wt = ((1.0 - alpha) * (W1.astype(np.float64) @ W2b.astype(np.float64))).astype(np.float32)
    b1w = (W2b.astype(np.float64).T @ b1.astype(np.float64))  # b1 @ W2b
    b3_full = np.tile(b3[None, :], (P, 1)).astype(np.float32)

    core_edge = edges // EPC
    core_vert = vertex // VPC

    EPCP = NW2 * P  # padded (relabeled) local edge id space
    VPCP = NW3 * P  # padded (relabeled) local vertex id space
    H = (EPCP // 2 // P) * P if EPCP >= 2 * P else max(EPCP // 2, 1)

    # per-core relabeling so every 128-wide window has a balanced pair count
    eperm, vperm = [], []
    for i in range(ncores):
        eperm.append(_balance_bins(edeg[i * EPC:(i + 1) * EPC].astype(np.int64), NW2, P))
        vperm.append(_balance_bins(deg[i * VPC:(i + 1) * VPC].astype(np.int64), NW3, P))
    eperm_all = np.concatenate(eperm)  # indexed by global edge id

    # AllGather piece boundaries over the padded local edge space
    HB = [0, H, EPCP] if EPCP >= 2 * P else [0, EPCP]
    NP3 = len(HB) - 1

    # ---- per-core pair lists
    s2lo_w, s2hi_w = [], []
    s2lo_pairs, s2hi_pairs = [], []
    s3_w = [[] for _ in range(NP3)]
    s3_pairs = [[] for _ in range(NP3)]
    for i in range(ncores):
        sel = np.nonzero(core_edge == i)[0]
        pv = vertex[sel]
        pe = eperm[i][edges[sel] - i * EPC]
        mlo = pv < LO
        for store_w, store_p, v, e in (
            (s2lo_w, s2lo_pairs, pv[mlo], pe[mlo]),
            (s2hi_w, s2hi_pairs, pv[~mlo] - LO, pe[~mlo]),
        ):
            store_w.append(np.bincount(e // P, minlength=NW2))
            store_p.append((e, v))
        sel = np.nonzero(core_vert == i)[0]
        pe = edges[sel]
        pvl = vperm[i][vertex[sel] - i * VPC]
        el = eperm_all[pe]
        er = pe // EPC
        # split by AllGather piece; gather idx into each piece's layout
        for k in range(NP3):
            mk = (el >= HB[k]) & (el < HB[k + 1])
            ik = er[mk] * (HB[k + 1] - HB[k]) + (el[mk] - HB[k])
            s3_w[k].append(np.bincount(pvl[mk] // P, minlength=NW3))
            s3_pairs[k].append((pvl[mk], ik))

    C2lo = _chunk_counts(s2lo_w, NW2, G)
    C2hi = _chunk_counts(s2hi_w, NW2, G)
    C3 = [_chunk_counts(s3_w[k], NW3, G, min_one=(k == 0)) for k in range(NP3)]
    S2LO, S2HI = int(C2lo.sum()), int(C2hi.sum())
    S3 = [int(c.sum()) for c in C3]

    iota = np.tile(np.arange(P, dtype=np.float32), G)[None, :].repeat(P, axis=0)
    iota = np.ascontiguousarray(iota)

    import ml_dtypes
    X_bf16 = X.astype(ml_dtypes.bfloat16)
    iota = iota.astype(ml_dtypes.bfloat16)
    W3h = W3.astype(ml_dtypes.bfloat16)

    in_maps = []
    for i in range(ncores):
        e, v = s2lo_pairs[i]
        lo_idx, lo_seg = _pack_stream(e, v, NW2, C2lo)
        e, v = s2hi_pairs[i]
        hi_idx, hi_seg = _pack_stream(e, v, NW2, C2hi)
        s3_packed = []
        for k in range(NP3):
            pvl, ik = s3_pairs[k][i]
            s3_packed.append(_pack_stream(pvl, ik, NW3, C3[k]))

        sl = slice(i * VPC, (i + 1) * VPC)
        xd_full = np.zeros((VPCP, D), dtype=np.float64)
        xd_full[vperm[i]] = X[sl].astype(np.float64) * deg[sl, None]
        xd_t = np.ascontiguousarray(xd_full.T).astype(np.float32)
        x0h = alpha * X0[sl].astype(np.float64).T \
            + (1.0 - alpha) * (np.outer(b2, deg[sl]) + np.outer(b1w, wdeg[sl]))
        x0h_full = np.zeros((D, VPCP), dtype=np.float64)
        x0h_full[:, vperm[i]] = x0h
        x0h_t = np.ascontiguousarray(x0h_full).astype(np.float32)

        im = {
            "x_tab": X_bf16,
            "s2lo_idx": _wrap_idx(lo_idx, G), "s2lo_seg": _seg_tile(lo_seg),
            "s2hi_idx": _wrap_idx(hi_idx, G), "s2hi_seg": _seg_tile(hi_seg),
            "iota": iota,
            "xd_t": xd_t,
            "x0h_t": x0h_t,
            "wa": Wa, "wt": Wt, "w3": W3h, "b3f": b3_full,
        }
        for k in range(NP3):
            pidx, pseg = s3_packed[k]
            im[f"s3p{k}_idx"] = _wrap_idx(pidx, G)
            im[f"s3p{k}_seg"] = _seg_tile(pseg)
        in_maps.append(im)

    meta = dict(N=N, M=M, NNZ=NNZ, ncores=ncores, G=G, LO=LO, H=H,
                EPC=EPC, VPC=VPC, NW2=NW2, NW3=NW3, EPCP=EPCP, VPCP=VPCP,
                HB=HB, NP3=NP3,
                C2lo=C2lo.tolist(), C2hi=C2hi.tolist(),
                C3=[c.tolist() for c in C3],
                S2LO=S2LO, S2HI=S2HI, S3=S3)
    meta["vperm"] = vperm
    return in_maps, meta


# ---------------------------------------------------------------------------
# device program
# ---------------------------------------------------------------------------

def build_program(meta):
    import concourse.bacc as bacc
    import concourse.bass as bass  # noqa: F401
    import concourse.mybir as mybir
    import concourse.tile as tile
    from concourse._compat import get_trn_type
    from concourse import library_config
    from concourse.tile_rust import add_dep_helper

    f32 = mybir.dt.float32
    bf16 = mybir.dt.bfloat16
    i16 = mybir.dt.int16

    ncores = meta["ncores"]
    G = meta["G"]
    N, M = meta["N"], meta["M"]
    LO = meta["LO"]
    EPC, VPC = meta["EPCP"], meta["VPCP"]
    NW2, NW3 = meta["NW2"], meta["NW3"]
    C2lo, C2hi = meta["C2lo"], meta["C2hi"]
    C3 = meta["C3"]
    S2LO, S2HI = meta["S2LO"], meta["S2HI"]
    S3 = meta["S3"]
    HB, NP3 = meta["HB"], meta["NP3"]
    H = meta["H"]
    GP = G * P

    nc = bacc.Bacc(get_trn_type() or "TRN2", num_devices=ncores, num_swdge_queues=4,
                   dynamic_dma_scratch_size=32768)

    x_tab = nc.declare_dram_parameter("x_tab", [N, D], bf16, isOutput=False)
    s2lo_idx = nc.declare_dram_parameter("s2lo_idx", [P, S2LO * 8], i16, isOutput=False)
    s2lo_seg = nc.declare_dram_parameter("s2lo_seg", [P, S2LO], bf16, isOutput=False)
    s2hi_idx = nc.declare_dram_parameter("s2hi_idx", [P, S2HI * 8], i16, isOutput=False)
    s2hi_seg = nc.declare_dram_parameter("s2hi_seg", [P, S2HI], bf16, isOutput=False)
    s3_idx_d, s3_seg_d = [], []
    for k in range(NP3):
        s3_idx_d.append(nc.declare_dram_parameter(f"s3p{k}_idx", [P, S3[k] * 8], i16, isOutput=False))
        s3_seg_d.append(nc.declare_dram_parameter(f"s3p{k}_seg", [P, S3[k]], bf16, isOutput=False))
    iota_d = nc.declare_dram_parameter("iota", [P, GP], bf16, isOutput=False)
    xd_d = nc.declare_dram_parameter("xd_t", [D, VPC], f32, isOutput=False)
    x0h_d = nc.declare_dram_parameter("x0h_t", [D, VPC], f32, isOutput=False)
    wa_d = nc.declare_dram_parameter("wa", [D, D], f32, isOutput=False)
    wt_d = nc.declare_dram_parameter("wt", [D, D], f32, isOutput=False)
    w3_d = nc.declare_dram_parameter("w3", [D, D], bf16, isOutput=False)
    b3f_d = nc.declare_dram_parameter("b3f", [P, D], f32, isOutput=False)
    out_d = nc.declare_dram_parameter("out", [VPC, D], f32, isOutput=True)

    se_slice = nc.dram_tensor("se_slice", [EPC, D], bf16)
    se_p = [nc.dram_tensor(f"se_p{k}", [ncores * (HB[k + 1] - HB[k]), D], bf16,
                           addr_space="Shared") for k in range(NP3)]

    with tile.TileContext(nc) as tc:
        with (
            tc.tile_pool(name="consts", bufs=1) as consts,
            tc.tile_pool(name="resident", bufs=1) as resident,
            tc.tile_pool(name="gat", bufs=10) as gat,
            tc.tile_pool(name="ohp", bufs=10) as ohp,
            tc.tile_pool(name="sep", bufs=3) as sep,
            tc.tile_pool(name="winp", bufs=5, space="PSUM") as winp,
            tc.tile_pool(name="zvp", bufs=1, space="PSUM") as zvp,
            tc.tile_pool(name="outp", bufs=2, space="PSUM") as outp,
        ):
            # ---- resident loads
            iota_t = consts.tile([P, G, P], bf16)
            nc.sync.dma_start(iota_t[:], iota_d[:].rearrange("p (g q) -> p g q", q=P))
            wa_t = consts.tile([D, D], f32)
            nc.sync.dma_start(wa_t[:], wa_d[:])
            wt_t = consts.tile([D, D], f32)
            nc.sync.dma_start(wt_t[:], wt_d[:])
            w3_t = consts.tile([D, D], bf16)
            nc.sync.dma_start(w3_t[:], w3_d[:])
            b3f_t = consts.tile([P, D], f32)
            nc.sync.dma_start(b3f_t[:], b3f_d[:])

            nc.gpsimd.load_library(library_config.mlp)
            npairs_reg = nc.gpsimd.to_reg(GP)
            qctr = [0]

            class Stream:
                NB0 = 4  # batches in the small head tile (fast first load)

                def __init__(self, name, idx_d, seg_d, n_chunks, table_ap, counts,
                             split_head=False):
                    self.name = name
                    self.counts = counts
                    self.off = np.concatenate([[0], np.cumsum(counts)[:-1]]).astype(int)
                    self.table_ap = table_ap
                    nb = n_chunks // G
                    self.head_batches = min(self.NB0, nb) if split_head else 0
                    hc = self.head_batches * G
                    self.idx_parts = []
                    if hc:
                        t0 = resident.tile([P, hc * 8], i16, tag=f"idx0_{name}")
                        nc.sync.dma_start(t0[:], idx_d[:, :hc * 8])
                        self.idx_parts.append(t0)
                    if n_chunks > hc:
                        t1 = resident.tile([P, (n_chunks - hc) * 8], i16, tag=f"idx_{name}")
                        nc.sync.dma_start(t1[:], idx_d[:, hc * 8:])
                        self.idx_parts.append(t1)
                    self.seg_t = resident.tile([P, n_chunks], bf16, tag=f"seg_{name}")
                    nc.sync.dma_start(self.seg_t[:], seg_d[:])
                    self.batches = {}
                    self.gather_insts = []

                def _idx_slice(self, b):
                    if b < self.head_batches:
                        return self.idx_parts[0][:, b * G * 8:(b + 1) * G * 8]
                    b -= self.head_batches
                    return self.idx_parts[-1][:, b * G * 8:(b + 1) * G * 8]

                def batch(self, b):
                    if b not in self.batches:
                        gt = gat.tile([P, G, D], bf16, tag="gat")
                        inst = nc.gpsimd.dma_gather(
                            gt[:],
                            self.table_ap,
                            self._idx_slice(b),
                            GP,
                            npairs_reg,
                            D,
                            queue_num=qctr[0] % 4,
                        )
                        qctr[0] += 1
                        self.gather_insts.append(inst)
                        oh = ohp.tile([P, G, P], bf16, tag="oh")
                        nc.vector.tensor_tensor(
                            out=oh[:],
                            in0=iota_t[:],
                            in1=self.seg_t[:, b * G:(b + 1) * G].broadcast_to([P, G, P]),
                            op=mybir.AluOpType.is_equal,
                        )
                        self.batches[b] = (gt, oh)
                    return self.batches[b]

            lo = Stream("s2lo", s2lo_idx, s2lo_seg, S2LO, x_tab[0:LO, :], C2lo,
                        split_head=True)
            streams2 = [lo]
            if LO < N:
                hi = Stream("s2hi", s2hi_idx, s2hi_seg, S2HI, x_tab[LO:N, :], C2hi,
                            split_head=True)
                streams2.append(hi)

            # ---- stage A: Se[e] = sum_{pairs with edge e} X[v]
            # AllGather fires per piece as its windows finish flushing.
            flushes = [[] for _ in range(NP3)]
            ags = [None] * NP3
            w_ag = [HB[k + 1] // P - 1 for k in range(NP3)]
            for w in range(NW2):
                total_k = sum(int(s.counts[w]) for s in streams2)
                psum_w = winp.tile([P, P], f32, tag="win")
                k = 0
                for s in streams2:
                    for c in range(int(s.off[w]), int(s.off[w]) + int(s.counts[w])):
                        b, cl = divmod(c, G)
                        gt, oh = s.batch(b)
                        nc.tensor.matmul(
                            psum_w[:],
                            lhsT=oh[:, cl, :],
                            rhs=gt[:, cl, :],
                            start=(k == 0),
                            stop=(k == total_k - 1),
                        )
                        k += 1
                st = sep.tile([P, P], bf16, tag="seflush")
                nc.vector.tensor_copy(out=st[:], in_=psum_w[:])
                fl = nc.sync.dma_start(out=se_slice[w * P:(w + 1) * P, :], in_=st[:])
                kp = next(k2 for k2 in range(NP3) if w * P < HB[k2 + 1])
                flushes[kp].append(fl)
                if w == w_ag[kp]:
                    ags[kp] = nc.gpsimd.collective_compute(
                        "AllGather", mybir.AluOpType.bypass,
                        replica_groups=[list(range(ncores))],
                        ins=[se_slice[HB[kp]:HB[kp + 1], :]], outs=[se_p[kp][:]])
                    for f in flushes[kp]:
                        add_dep_helper(ags[kp].ins, f.ins,
                                       reason=f"AG{kp} reads its se_slice piece")

            # ---- stage B: T[v] = sum_{pairs with vertex v} Se[e]
            # one pass per AllGather piece; the dense tail (stages C/D) is
            # interleaved as windows finalize during the last pass.
            streams3 = [Stream(f"s3p{k}", s3_idx_d[k], s3_seg_d[k], S3[k],
                               se_p[k][:], C3[k]) for k in range(NP3)]
            Tt = resident.tile([P, NW3 * P], f32, tag="Tt")
            xd_t = resident.tile([D, VPC], f32, tag="xd")
            nc.sync.dma_start(xd_t[:], xd_d[:])
            x0h_t = resident.tile([D, VPC], f32, tag="x0h")
            nc.sync.dma_start(x0h_t[:], x0h_d[:])
            zt_t = resident.tile([D, VPC], bf16, tag="zt")

            RT = 512

            def emit_c_tile(rt):
                s0 = rt * RT
                L = min(RT, VPC - s0)
                pz = zvp.tile([P, RT], f32, tag="zv")
                nc.tensor.matmul(pz[:, :L], lhsT=wa_t[:], rhs=xd_t[:, s0:s0 + L],
                                 start=True, stop=False)
                nc.tensor.matmul(pz[:, :L], lhsT=wt_t[:], rhs=Tt[:, s0:s0 + L],
                                 start=False, stop=True)
                nc.vector.tensor_add(out=zt_t[:, s0:s0 + L], in0=pz[:, :L],
                                     in1=x0h_t[:, s0:s0 + L])
                for ot in range(s0 // P, (s0 + L + P - 1) // P):
                    o0 = ot * P
                    Lo = min(P, VPC - o0)
                    po = outp.tile([P, P], f32, tag="out")
                    nc.tensor.matmul(po[:Lo, :], lhsT=zt_t[:, o0:o0 + Lo], rhs=w3_t[:],
                                     start=True, stop=True)
                    st = sep.tile([P, P], f32, tag="outflush")
                    nc.vector.tensor_tensor(out=st[:Lo, :], in0=po[:Lo, :],
                                            in1=b3f_t[:Lo, :], op=mybir.AluOpType.add)
                    nc.sync.dma_start(out=out_d[o0:o0 + Lo, :], in_=st[:Lo, :])

            n_ctiles = (VPC + RT - 1) // RT
            for kp, s3 in enumerate(streams3):
                last = kp == NP3 - 1
                done_c = 0
                for w in range(NW3):
                    total_k = int(s3.counts[w])
                    if total_k > 0:
                        psum_w = winp.tile([P, P], f32, tag="win")
                        for k, c in enumerate(range(int(s3.off[w]), int(s3.off[w]) + total_k)):
                            b, cl = divmod(c, G)
                            gt, oh = s3.batch(b)
                            nc.tensor.matmul(
                                psum_w[:],
                                lhsT=gt[:, cl, :],
                                rhs=oh[:, cl, :],
                                start=(k == 0),
                                stop=(k == total_k - 1),
                            )
                        if kp == 0:
                            nc.vector.tensor_copy(out=Tt[:, w * P:(w + 1) * P], in_=psum_w[:])
                        else:
                            nc.vector.tensor_add(out=Tt[:, w * P:(w + 1) * P],
                                                 in0=Tt[:, w * P:(w + 1) * P], in1=psum_w[:])
                    # emit any C tiles fully covered by finalized windows
                    if last:
                        while done_c < n_ctiles and (done_c + 1) * RT <= (w + 1) * P:
                            emit_c_tile(done_c)
                            done_c += 1
                if last:
                    while done_c < n_ctiles:
                        emit_c_tile(done_c)
                        done_c += 1

            for kp, s3 in enumerate(streams3):
                for inst in s3.gather_insts:
                    add_dep_helper(inst.ins, ags[kp].ins,
                                   reason=f"pass-{kp} gathers read se_p{kp}")

    return nc


# ---------------------------------------------------------------------------
# entry point
# ---------------------------------------------------------------------------

def _run(inputs, trace=False, M=25000, ncores=8, G=8, lo_split=32768):
    import sys
    if "/opt/trn_rl_repo" not in sys.path:
        sys.path.insert(0, "/opt/trn_rl_repo")
    from concourse.bass_utils import run_bass_kernel_spmd

    in_maps, meta = preprocess(**inputs, M=M, ncores=ncores, G=G, lo_split=lo_split)
    nc = build_program(meta)
    if not nc.is_finalized():
        nc.finalize()
    res = run_bass_kernel_spmd(nc, in_maps, list(range(ncores)), trace=trace)
    vperm = meta["vperm"]
    out = np.concatenate(
        [np.asarray(res.results[i]["out"])[vperm[i]] for i in range(ncores)], axis=0)
    return out, res


def kernel(**inputs):
    out, _ = _run(inputs)
    return out



# revision 27
# speedup vs baseline: 1.0943x; 1.0943x over previous
"""Trainium2 Bass kernel for nn_EquivSetConv (hypergraph message passing).

Reference computation:
    Xve = (X @ W1 + b1)[vertex]
    Xe  = segment_sum(Xve, edges, M)
    Xev = Xe[edges]
    H   = concat([X[vertex], Xev], -1) @ W2 + b2
    Xv  = segment_sum(H, vertex, N)
    out = ((1-a)*Xv + a*X0) @ W3 + b3

Algebraic restructure (A[v,e] = #incidence pairs (v,e)):
    Se  = A^T @ X                          (segmented sum of raw X rows per edge)
    Xe  = Se @ W1 + edeg x b1
    T   = A @ Se                           (segmented sum of Se rows per vertex)
    Xv  = deg . (X @ W2a) + T @ (W1 @ W2b) + deg x b2 + wdeg x (b1 @ W2b)
    out = ((1-a)Xv + a X0) @ W3 + b3

So the 800k-row dense matmul disappears; the kernel is two sparse
gather+segmented-sum stages plus small dense matmuls.

Sharding over 8 cores: stage A partitioned by edge range (each core owns
M/8 edges and all pairs incident to them -> computes its Se slice fully,
no cross-core reduction), one AllGather of Se, stage B partitioned by
vertex range (each core owns N/8 vertices -> computes its output rows
end to end). The only collective is the 1.6MB/rank AllGather.

Sparse stages on device: host sorts pairs by destination segment and
packs them into 128-pair chunks that are pure in a 128-wide segment
window.  For each chunk: dma_gather 128 source rows (pair p -> SBUF
partition p), build a one-hot [pair, segment-slot] matrix on DVE
(iota == seg), and accumulate with one PE matmul into the window's PSUM
tile.  Windows flush to SBUF/DRAM when complete.
"""

import numpy as np

P = 128
D = 128


# ---------------------------------------------------------------------------
# host-side preprocessing
# ---------------------------------------------------------------------------

def _pack_stream(seg_local, gidx, n_windows, chunk_counts):
    """Pack pairs (sorted by window) into window-pure 128-slot chunks.

    seg_local: [n] int, segment id LOCAL to the stream's window grid
               (seg_local // 128 = window, seg_local % 128 = slot)
    gidx:      [n] int, gather index of each pair
    chunk_counts: [n_windows] int, chunks allocated per window (shared
               across all cores so the program structure is identical).

    Returns (idx16, segf) flat arrays of length sum(chunk_counts)*128,
    pad slots have idx 0 / seg -1.
    """
    total_chunks = int(np.sum(chunk_counts))
    tot = total_chunks * P
    idx16 = np.zeros(tot, dtype=np.int16)
    segf = np.full(tot, -1.0, dtype=np.float32)
    if len(seg_local) == 0:
        return idx16, segf

    order = np.argsort(seg_local, kind="stable")
    seg_s = seg_local[order]
    gidx_s = gidx[order]
    win = seg_s // P

    # position of each pair: chunk_base[win]*128 + rank-within-window
    chunk_base = np.concatenate([[0], np.cumsum(chunk_counts)[:-1]])
    win_start = np.searchsorted(win, np.arange(n_windows), side="left")
    rank = np.arange(len(win)) - win_start[win]
    pos = chunk_base[win] * P + rank
    idx16[pos] = gidx_s.astype(np.int16)
    segf[pos] = (seg_s % P).astype(np.float32)
    return idx16, segf


def _wrap_idx(idx16, G):
    """Reshape a flat per-stream idx array into the dma_gather SBUF layout.

    Within each batch of G*128 indices, index i lives at
    [partition i%16, column i//16]; batches are side by side (the last
    batch may cover fewer than G chunks; it is zero-padded for layout).
    Output [128, ceil(total_chunks/G)*G*8] int16 (rows 0..15 replicated).
    """
    rem = (-len(idx16)) % (G * P)
    if rem:
        idx16 = np.concatenate([idx16, np.zeros(rem, dtype=np.int16)])
    nb = len(idx16) // (G * P)
    blocks = [idx16[b * G * P:(b + 1) * G * P].reshape(G * 8, 16).T for b in range(nb)]
    arr16 = np.hstack(blocks)  # [16, nb*G*8]
    return np.tile(arr16, (8, 1)).astype(np.int16)


def _seg_tile(segf):
    """[total_chunks*128] -> [128, total_chunks]: pair (chunk c, part p)."""
    import ml_dtypes
    n_chunks = len(segf) // P
    return np.ascontiguousarray(segf.reshape(n_chunks, P).T).astype(ml_dtypes.bfloat16)


def _chunk_counts(windows_per_core, n_windows, G, min_one=True):
    """windows_per_core: list over cores of [n_windows] pair counts.
    Returns per-window chunk counts (max over cores), padded so the
    total is a multiple of G."""
    counts = np.zeros(n_windows, dtype=np.int64)
    for wc in windows_per_core:
        counts = np.maximum(counts, (wc + P - 1) // P)
    if min_one:
        counts = np.maximum(counts, 1)
    tot = int(counts.sum())
    counts[-1] += (-tot) % G
    return counts


def _balance_bins(loads, n_bins, cap):
    """LPT greedy: assign items (with given loads) to n_bins bins of
    capacity `cap` items each, minimizing max bin load.
    Returns [len(loads)] -> new id (bin*cap + slot-within-bin)."""
    import heapq
    order = np.argsort(-loads, kind="stable")
    heap = [(0, b) for b in range(n_bins)]
    heapq.heapify(heap)
    fill = np.zeros(n_bins, dtype=np.int64)
    newid = np.empty(len(loads), dtype=np.int64)
    spill = []
    for it in order:
        load, b = heapq.heappop(heap)
        newid[it] = b * cap + fill[b]
        fill[b] += 1
        if fill[b] < cap:
            heapq.heappush(heap, (load + loads[it], b))
        else:
            spill.append((load + loads[it], b))
    return newid


def preprocess(X, vertex, edges, X0, W1, b1, W2, b2, W3, b3,
               M=25000, ncores=8, G=8, lo_split=32768):
    """Build per-core input maps + compile-time metadata."""
    X = np.asarray(X, dtype=np.float32)
    X0 = np.asarray(X0, dtype=np.float32)
    vertex = np.asarray(vertex).astype(np.int64)
    edges = np.asarray(edges).astype(np.int64)
    W1 = np.asarray(W1, dtype=np.float32)
    b1 = np.asarray(b1, dtype=np.float32)
    W2 = np.asarray(W2, dtype=np.float32)
    b2 = np.asarray(b2, dtype=np.float32)
    W3 = np.asarray(W3, dtype=np.float32)
    b3 = np.asarray(b3, dtype=np.float32)

    N, Din = X.shape
    NNZ = len(vertex)
    Dout = W3.shape[1]
    assert Din == D and Dout == D

    LO = min(lo_split, N)
    EPC = M // ncores
    VPC = N // ncores
    assert M % ncores == 0 and N % ncores == 0
    NW2 = (EPC + P - 1) // P
    NW3 = (VPC + P - 1) // P

    alpha = 0.5
    W2a = W2[:D]
    W2b = W2[D:]
    deg = np.bincount(vertex, minlength=N).astype(np.float64)
    edeg = np.bincount(edges, minlength=M).astype(np.float64)
    wdeg = np.bincount(vertex, weights=edeg[edges], minlength=N)

    Wa = ((1.0 - alpha) * W2a).astype(np.float32)
    Wt = ((1.0 - alpha) * (W1.astype(np.float64) @ W2b.astype(np.float64))).astype(np.float32)
    b1w = (W2b.astype(np.float64).T @ b1.astype(np.float64))  # b1 @ W2b
    b3_full = np.tile(b3[None, :], (P, 1)).astype(np.float32)

    core_edge = edges // EPC
    core_vert = vertex // VPC

    EPCP = NW2 * P  # padded (relabeled) local edge id space
    VPCP = NW3 * P  # padded (relabeled) local vertex id space
    H = (EPCP // 2 // P) * P if EPCP >= 2 * P else max(EPCP // 2, 1)

    # per-core relabeling so every 128-wide window has a balanced pair count
    eperm, vperm = [], []
    for i in range(ncores):
        eperm.append(_balance_bins(edeg[i * EPC:(i + 1) * EPC].astype(np.int64), NW2, P))
        vperm.append(_balance_bins(deg[i * VPC:(i + 1) * VPC].astype(np.int64), NW3, P))
    eperm_all = np.concatenate(eperm)  # indexed by global edge id

    # AllGather piece boundaries over the padded local edge space
    HB = [0, H, EPCP] if EPCP >= 2 * P else [0, EPCP]
    NP3 = len(HB) - 1

    # ---- per-core pair lists
    s2lo_w, s2hi_w = [], []
    s2lo_pairs, s2hi_pairs = [], []
    s3_w = [[] for _ in range(NP3)]
    s3_pairs = [[] for _ in range(NP3)]
    for i in range(ncores):
        sel = np.nonzero(core_edge == i)[0]
        pv = vertex[sel]
        pe = eperm[i][edges[sel] - i * EPC]
        mlo = pv < LO
        for store_w, store_p, v, e in (
            (s2lo_w, s2lo_pairs, pv[mlo], pe[mlo]),
            (s2hi_w, s2hi_pairs, pv[~mlo] - LO, pe[~mlo]),
        ):
            store_w.append(np.bincount(e // P, minlength=NW2))
            store_p.append((e, v))
        sel = np.nonzero(core_vert == i)[0]
        pe = edges[sel]
        pvl = vperm[i][vertex[sel] - i * VPC]
        el = eperm_all[pe]
        er = pe // EPC
        # split by AllGather piece; gather idx into each piece's layout
        for k in range(NP3):
            mk = (el >= HB[k]) & (el < HB[k + 1])
            ik = er[mk] * (HB[k + 1] - HB[k]) + (el[mk] - HB[k])
            s3_w[k].append(np.bincount(pvl[mk] // P, minlength=NW3))
            s3_pairs[k].append((pvl[mk], ik))

    C2lo = _chunk_counts(s2lo_w, NW2, G)
    C2hi = _chunk_counts(s2hi_w, NW2, G)
    C3 = [_chunk_counts(s3_w[k], NW3, G, min_one=(k == 0)) for k in range(NP3)]
    S2LO, S2HI = int(C2lo.sum()), int(C2hi.sum())
    S3 = [int(c.sum()) for c in C3]

    iota = np.tile(np.arange(P, dtype=np.float32), G)[None, :].repeat(P, axis=0)
    iota = np.ascontiguousarray(iota)

    import ml_dtypes
    X_bf16 = X.astype(ml_dtypes.bfloat16)
    iota = iota.astype(ml_dtypes.bfloat16)
    W3h = W3.astype(ml_dtypes.bfloat16)

    in_maps = []
    for i in range(ncores):
        e, v = s2lo_pairs[i]
        lo_idx, lo_seg = _pack_stream(e, v, NW2, C2lo)
        e, v = s2hi_pairs[i]
        hi_idx, hi_seg = _pack_stream(e, v, NW2, C2hi)
        s3_packed = []
        for k in range(NP3):
            pvl, ik = s3_pairs[k][i]
            s3_packed.append(_pack_stream(pvl, ik, NW3, C3[k]))

        sl = slice(i * VPC, (i + 1) * VPC)
        xd_full = np.zeros((VPCP, D), dtype=np.float64)
        xd_full[vperm[i]] = X[sl].astype(np.float64) * deg[sl, None]
        xd_t = np.ascontiguousarray(xd_full.T).astype(np.float32)
        x0h = alpha * X0[sl].astype(np.float64).T \
            + (1.0 - alpha) * (np.outer(b2, deg[sl]) + np.outer(b1w, wdeg[sl]))
        x0h_full = np.zeros((D, VPCP), dtype=np.float64)
        x0h_full[:, vperm[i]] = x0h
        x0h_t = np.ascontiguousarray(x0h_full).astype(np.float32)

        im = {
            "x_tab": X_bf16,
            "s2lo_idx": _wrap_idx(lo_idx, G), "s2lo_seg": _seg_tile(lo_seg),
            "s2hi_idx": _wrap_idx(hi_idx, G), "s2hi_seg": _seg_tile(hi_seg),
            "iota": iota,
            "xd_t": xd_t,
            "x0h_t": x0h_t,
            "wa": Wa, "wt": Wt, "w3": W3h, "b3f": b3_full,
        }
        for k in range(NP3):
            pidx, pseg = s3_packed[k]
            im[f"s3p{k}_idx"] = _wrap_idx(pidx, G)
            im[f"s3p{k}_seg"] = _seg_tile(pseg)
        in_maps.append(im)

    meta = dict(N=N, M=M, NNZ=NNZ, ncores=ncores, G=G, LO=LO, H=H,
                EPC=EPC, VPC=VPC, NW2=NW2, NW3=NW3, EPCP=EPCP, VPCP=VPCP,
                HB=HB, NP3=NP3,
                C2lo=C2lo.tolist(), C2hi=C2hi.tolist(),
                C3=[c.tolist() for c in C3],
                S2LO=S2LO, S2HI=S2HI, S3=S3)
    meta["vperm"] = vperm
    return in_maps, meta


# ---------------------------------------------------------------------------
# device program
# ---------------------------------------------------------------------------

def build_program(meta):
    import concourse.bacc as bacc
    import concourse.bass as bass  # noqa: F401
    import concourse.mybir as mybir
    import concourse.tile as tile
    from concourse._compat import get_trn_type
    from concourse import library_config
    from concourse.tile_rust import add_dep_helper

    f32 = mybir.dt.float32
    bf16 = mybir.dt.bfloat16
    i16 = mybir.dt.int16

    ncores = meta["ncores"]
    G = meta["G"]
    N, M = meta["N"], meta["M"]
    LO = meta["LO"]
    EPC, VPC = meta["EPCP"], meta["VPCP"]
    NW2, NW3 = meta["NW2"], meta["NW3"]
    C2lo, C2hi = meta["C2lo"], meta["C2hi"]
    C3 = meta["C3"]
    S2LO, S2HI = meta["S2LO"], meta["S2HI"]
    S3 = meta["S3"]
    HB, NP3 = meta["HB"], meta["NP3"]
    H = meta["H"]
    GP = G * P

    nc = bacc.Bacc(get_trn_type() or "TRN2", num_devices=ncores, num_swdge_queues=4,
                   dynamic_dma_scratch_size=32768)

    x_tab = nc.declare_dram_parameter("x_tab", [N, D], bf16, isOutput=False)
    s2lo_idx = nc.declare_dram_parameter("s2lo_idx", [P, S2LO * 8], i16, isOutput=False)
    s2lo_seg = nc.declare_dram_parameter("s2lo_seg", [P, S2LO], bf16, isOutput=False)
    s2hi_idx = nc.declare_dram_parameter("s2hi_idx", [P, S2HI * 8], i16, isOutput=False)
    s2hi_seg = nc.declare_dram_parameter("s2hi_seg", [P, S2HI], bf16, isOutput=False)
    s3_idx_d, s3_seg_d = [], []
    for k in range(NP3):
        s3_idx_d.append(nc.declare_dram_parameter(f"s3p{k}_idx", [P, S3[k] * 8], i16, isOutput=False))
        s3_seg_d.append(nc.declare_dram_parameter(f"s3p{k}_seg", [P, S3[k]], bf16, isOutput=False))
    iota_d = nc.declare_dram_parameter("iota", [P, GP], bf16, isOutput=False)
    xd_d = nc.declare_dram_parameter("xd_t", [D, VPC], f32, isOutput=False)
    x0h_d = nc.declare_dram_parameter("x0h_t", [D, VPC], f32, isOutput=False)
    wa_d = nc.declare_dram_parameter("wa", [D, D], f32, isOutput=False)
    wt_d = nc.declare_dram_parameter("wt", [D, D], f32, isOutput=False)
    w3_d = nc.declare_dram_parameter("w3", [D, D], bf16, isOutput=False)
    b3f_d = nc.declare_dram_parameter("b3f", [P, D], f32, isOutput=False)
    out_d = nc.declare_dram_parameter("out", [VPC, D], f32, isOutput=True)

    se_slice = nc.dram_tensor("se_slice", [EPC, D], bf16)
    se_p = [nc.dram_tensor(f"se_p{k}", [ncores * (HB[k + 1] - HB[k]), D], bf16,
                           addr_space="Shared") for k in range(NP3)]

    with tile.TileContext(nc) as tc:
        with (
            tc.tile_pool(name="consts", bufs=1) as consts,
            tc.tile_pool(name="resident", bufs=1) as resident,
            tc.tile_pool(name="gat", bufs=10) as gat,
            tc.tile_pool(name="ohp", bufs=10) as ohp,
            tc.tile_pool(name="sep", bufs=3) as sep,
            tc.tile_pool(name="winp", bufs=5, space="PSUM") as winp,
            tc.tile_pool(name="zvp", bufs=1, space="PSUM") as zvp,
            tc.tile_pool(name="outp", bufs=2, space="PSUM") as outp,
        ):
            # ---- resident loads
            iota_t = consts.tile([P, G, P], bf16)
            nc.sync.dma_start(iota_t[:], iota_d[:].rearrange("p (g q) -> p g q", q=P))
            wa_t = consts.tile([D, D], f32)
            nc.sync.dma_start(wa_t[:], wa_d[:])
            wt_t = consts.tile([D, D], f32)
            nc.sync.dma_start(wt_t[:], wt_d[:])
            w3_t = consts.tile([D, D], bf16)
            nc.sync.dma_start(w3_t[:], w3_d[:])
            b3f_t = consts.tile([P, D], f32)
            nc.sync.dma_start(b3f_t[:], b3f_d[:])

            nc.gpsimd.load_library(library_config.mlp)
            npairs_reg = nc.gpsimd.to_reg(GP)
            qctr = [0]

            class Stream:
                def __init__(self, name, idx_d, seg_d, n_chunks, table_ap, counts,
                             split_head=False):
                    self.name = name
                    self.counts = counts
                    self.off = np.concatenate([[0], np.cumsum(counts)[:-1]]).astype(int)
                    self.table_ap = table_ap
                    self.idx_t = resident.tile([P, n_chunks * 8], i16, tag=f"idx_{name}")
                    nc.sync.dma_start(self.idx_t[:], idx_d[:])
                    self.seg_t = resident.tile([P, n_chunks], bf16, tag=f"seg_{name}")
                    nc.sync.dma_start(self.seg_t[:], seg_d[:])
                    self.batches = {}
                    self.gather_insts = []

                def batch(self, b):
                    if b not in self.batches:
                        gt = gat.tile([P, G, D], bf16, tag="gat")
                        inst = nc.gpsimd.dma_gather(
                            gt[:],
                            self.table_ap,
                            self.idx_t[:, b * G * 8:(b + 1) * G * 8],
                            GP,
                            npairs_reg,
                            D,
                            queue_num=qctr[0] % 4,
                        )
                        qctr[0] += 1
                        self.gather_insts.append(inst)
                        oh = ohp.tile([P, G, P], bf16, tag="oh")
                        nc.vector.tensor_tensor(
                            out=oh[:],
                            in0=iota_t[:],
                            in1=self.seg_t[:, b * G:(b + 1) * G].broadcast_to([P, G, P]),
                            op=mybir.AluOpType.is_equal,
                        )
                        self.batches[b] = (gt, oh)
                    return self.batches[b]

            lo = Stream("s2lo", s2lo_idx, s2lo_seg, S2LO, x_tab[0:LO, :], C2lo,
                        split_head=True)
            streams2 = [lo]
            if LO < N:
                hi = Stream("s2hi", s2hi_idx, s2hi_seg, S2HI, x_tab[LO:N, :], C2hi,
                            split_head=True)
                streams2.append(hi)

            # ---- stage A: Se[e] = sum_{pairs with edge e} X[v]
            # AllGather fires per piece as its windows finish flushing.
            flushes = [[] for _ in range(NP3)]
            ags = [None] * NP3
            w_ag = [HB[k + 1] // P - 1 for k in range(NP3)]
            for w in range(NW2):
                total_k = sum(int(s.counts[w]) for s in streams2)
                psum_w = winp.tile([P, P], f32, tag="win")
                k = 0
                for s in streams2:
                    for c in range(int(s.off[w]), int(s.off[w]) + int(s.counts[w])):
                        b, cl = divmod(c, G)
                        gt, oh = s.batch(b)
                        nc.tensor.matmul(
                            psum_w[:],
                            lhsT=oh[:, cl, :],
                            rhs=gt[:, cl, :],
                            start=(k == 0),
                            stop=(k == total_k - 1),
                        )
                        k += 1
                st = sep.tile([P, P], bf16, tag="seflush")
                nc.vector.tensor_copy(out=st[:], in_=psum_w[:])
                fl = nc.sync.dma_start(out=se_slice[w * P:(w + 1) * P, :], in_=st[:])
                kp = next(k2 for k2 in range(NP3) if w * P < HB[k2 + 1])
                flushes[kp].append(fl)
                if w == w_ag[kp]:
                    ags[kp] = nc.gpsimd.collective_compute(
                        "AllGather", mybir.AluOpType.bypass,
                        replica_groups=[list(range(ncores))],
                        ins=[se_slice[HB[kp]:HB[kp + 1], :]], outs=[se_p[kp][:]])
                    for f in flushes[kp]:
                        add_dep_helper(ags[kp].ins, f.ins,
                                       reason=f"AG{kp} reads its se_slice piece")

            # ---- stage B: T[v] = sum_{pairs with vertex v} Se[e]
            # one pass per AllGather piece; the dense tail (stages C/D) is
            # interleaved as windows finalize during the last pass.
            streams3 = [Stream(f"s3p{k}", s3_idx_d[k], s3_seg_d[k], S3[k],
                               se_p[k][:], C3[k]) for k in range(NP3)]
            Tt = resident.tile([P, NW3 * P], f32, tag="Tt")
            xd_t = resident.tile([D, VPC], f32, tag="xd")
            nc.sync.dma_start(xd_t[:], xd_d[:])
            x0h_t = resident.tile([D, VPC], f32, tag="x0h")
            nc.sync.dma_start(x0h_t[:], x0h_d[:])
            zt_t = resident.tile([D, VPC], bf16, tag="zt")

            RT = 512

            def emit_c_tile(rt):
                s0 = rt * RT
                L = min(RT, VPC - s0)
                pz = zvp.tile([P, RT], f32, tag="zv")
                nc.tensor.matmul(pz[:, :L], lhsT=wa_t[:], rhs=xd_t[:, s0:s0 + L],
                                 start=True, stop=False)
                nc.tensor.matmul(pz[:, :L], lhsT=wt_t[:], rhs=Tt[:, s0:s0 + L],
                                 start=False, stop=True)
                nc.vector.tensor_add(out=zt_t[:, s0:s0 + L], in0=pz[:, :L],
                                     in1=x0h_t[:, s0:s0 + L])
                for ot in range(s0 // P, (s0 + L + P - 1) // P):
                    o0 = ot * P
                    Lo = min(P, VPC - o0)
                    po = outp.tile([P, P], f32, tag="out")
                    nc.tensor.matmul(po[:Lo, :], lhsT=zt_t[:, o0:o0 + Lo], rhs=w3_t[:],
                                     start=True, stop=True)
                    st = sep.tile([P, P], f32, tag="outflush")
                    nc.vector.tensor_tensor(out=st[:Lo, :], in0=po[:Lo, :],
                                            in1=b3f_t[:Lo, :], op=mybir.AluOpType.add)
                    nc.sync.dma_start(out=out_d[o0:o0 + Lo, :], in_=st[:Lo, :])

            n_ctiles = (VPC + RT - 1) // RT
            for kp, s3 in enumerate(streams3):
                last = kp == NP3 - 1
                done_c = 0
                for w in range(NW3):
                    total_k = int(s3.counts[w])
                    if total_k > 0:
                        psum_w = winp.tile([P, P], f32, tag="win")
                        for k, c in enumerate(range(int(s3.off[w]), int(s3.off[w]) + total_k)):
                            b, cl = divmod(c, G)
                            gt, oh = s3.batch(b)
                            nc.tensor.matmul(
                                psum_w[:],
                                lhsT=gt[:, cl, :],
                                rhs=oh[:, cl, :],
                                start=(k == 0),
                                stop=(k == total_k - 1),
                            )
                        if kp == 0:
                            nc.vector.tensor_copy(out=Tt[:, w * P:(w + 1) * P], in_=psum_w[:])
                        else:
                            nc.vector.tensor_add(out=Tt[:, w * P:(w + 1) * P],
                                                 in0=Tt[:, w * P:(w + 1) * P], in1=psum_w[:])
                    # emit any C tiles fully covered by finalized windows
                    if last:
                        while done_c < n_ctiles and (done_c + 1) * RT <= (w + 1) * P:
                            emit_c_tile(done_c)
                            done_c += 1
                if last:
                    while done_c < n_ctiles:
                        emit_c_tile(done_c)
                        done_c += 1

            for kp, s3 in enumerate(streams3):
                for inst in s3.gather_insts:
                    add_dep_helper(inst.ins, ags[kp].ins,
                                   reason=f"pass-{kp} gathers read se_p{kp}")

    return nc


# ---------------------------------------------------------------------------
# entry point
# ---------------------------------------------------------------------------

def _run(inputs, trace=False, M=25000, ncores=8, G=8, lo_split=32768):
    import sys
    if "/opt/trn_rl_repo" not in sys.path:
        sys.path.insert(0, "/opt/trn_rl_repo")
    from concourse.bass_utils import run_bass_kernel_spmd

    in_maps, meta = preprocess(**inputs, M=M, ncores=ncores, G=G, lo_split=lo_split)
    nc = build_program(meta)
    if not nc.is_finalized():
        nc.finalize()
    res = run_bass_kernel_spmd(nc, in_maps, list(range(ncores)), trace=trace)
    vperm = meta["vperm"]
    out = np.concatenate(
        [np.asarray(res.results[i]["out"])[vperm[i]] for i in range(ncores)], axis=0)
    return out, res


def kernel(**inputs):
    out, _ = _run(inputs)
    return out



# revision 28
# speedup vs baseline: 1.1388x; 1.0407x over previous
"""Trainium2 Bass kernel for nn_EquivSetConv (hypergraph message passing).

Reference computation:
    Xve = (X @ W1 + b1)[vertex]
    Xe  = segment_sum(Xve, edges, M)
    Xev = Xe[edges]
    H   = concat([X[vertex], Xev], -1) @ W2 + b2
    Xv  = segment_sum(H, vertex, N)
    out = ((1-a)*Xv + a*X0) @ W3 + b3

Algebraic restructure (A[v,e] = #incidence pairs (v,e)):
    Se  = A^T @ X                          (segmented sum of raw X rows per edge)
    Xe  = Se @ W1 + edeg x b1
    T   = A @ Se                           (segmented sum of Se rows per vertex)
    Xv  = deg . (X @ W2a) + T @ (W1 @ W2b) + deg x b2 + wdeg x (b1 @ W2b)
    out = ((1-a)Xv + a X0) @ W3 + b3

So the 800k-row dense matmul disappears; the kernel is two sparse
gather+segmented-sum stages plus small dense matmuls.

Sharding over 8 cores: stage A partitioned by edge range (each core owns
M/8 edges and all pairs incident to them -> computes its Se slice fully,
no cross-core reduction), one AllGather of Se, stage B partitioned by
vertex range (each core owns N/8 vertices -> computes its output rows
end to end). The only collective is the 1.6MB/rank AllGather.

Sparse stages on device: host sorts pairs by destination segment and
packs them into 128-pair chunks that are pure in a 128-wide segment
window.  For each chunk: dma_gather 128 source rows (pair p -> SBUF
partition p), build a one-hot [pair, segment-slot] matrix on DVE
(iota == seg), and accumulate with one PE matmul into the window's PSUM
tile.  Windows flush to SBUF/DRAM when complete.
"""

import numpy as np

P = 128
D = 128


# ---------------------------------------------------------------------------
# host-side preprocessing
# ---------------------------------------------------------------------------

def _pack_stream(seg_local, gidx, n_windows, chunk_counts):
    """Pack pairs (sorted by window) into window-pure 128-slot chunks.

    seg_local: [n] int, segment id LOCAL to the stream's window grid
               (seg_local // 128 = window, seg_local % 128 = slot)
    gidx:      [n] int, gather index of each pair
    chunk_counts: [n_windows] int, chunks allocated per window (shared
               across all cores so the program structure is identical).

    Returns (idx16, segf) flat arrays of length sum(chunk_counts)*128,
    pad slots have idx 0 / seg -1.
    """
    total_chunks = int(np.sum(chunk_counts))
    tot = total_chunks * P
    idx16 = np.zeros(tot, dtype=np.int16)
    segf = np.full(tot, -1.0, dtype=np.float32)
    if len(seg_local) == 0:
        return idx16, segf

    order = np.argsort(seg_local, kind="stable")
    seg_s = seg_local[order]
    gidx_s = gidx[order]
    win = seg_s // P

    # position of each pair: chunk_base[win]*128 + rank-within-window
    chunk_base = np.concatenate([[0], np.cumsum(chunk_counts)[:-1]])
    win_start = np.searchsorted(win, np.arange(n_windows), side="left")
    rank = np.arange(len(win)) - win_start[win]
    pos = chunk_base[win] * P + rank
    idx16[pos] = gidx_s.astype(np.int16)
    segf[pos] = (seg_s % P).astype(np.float32)
    return idx16, segf


def _wrap_idx(idx16, G):
    """Reshape a flat per-stream idx array into the dma_gather SBUF layout.

    Within each batch of G*128 indices, index i lives at
    [partition i%16, column i//16]; batches are side by side (the last
    batch may cover fewer than G chunks; it is zero-padded for layout).
    Output [128, ceil(total_chunks/G)*G*8] int16 (rows 0..15 replicated).
    """
    rem = (-len(idx16)) % (G * P)
    if rem:
        idx16 = np.concatenate([idx16, np.zeros(rem, dtype=np.int16)])
    nb = len(idx16) // (G * P)
    blocks = [idx16[b * G * P:(b + 1) * G * P].reshape(G * 8, 16).T for b in range(nb)]
    arr16 = np.hstack(blocks)  # [16, nb*G*8]
    return np.tile(arr16, (8, 1)).astype(np.int16)


def _seg_tile(segf):
    """[total_chunks*128] -> [128, total_chunks]: pair (chunk c, part p)."""
    import ml_dtypes
    n_chunks = len(segf) // P
    return np.ascontiguousarray(segf.reshape(n_chunks, P).T).astype(ml_dtypes.bfloat16)


def _chunk_counts(windows_per_core, n_windows, G, min_one=True):
    """windows_per_core: list over cores of [n_windows] pair counts.
    Returns per-window chunk counts (max over cores), padded so the
    total is a multiple of G."""
    counts = np.zeros(n_windows, dtype=np.int64)
    for wc in windows_per_core:
        counts = np.maximum(counts, (wc + P - 1) // P)
    if min_one:
        counts = np.maximum(counts, 1)
    tot = int(counts.sum())
    counts[-1] += (-tot) % G
    return counts


def _balance_bins(loads2d, n_bins, cap):
    """Greedy multi-dim LPT: assign items (rows of loads2d, one load per
    stream dimension) to n_bins bins of capacity `cap` items, keeping every
    per-dimension bin load balanced.  Returns [n] -> bin*cap + slot."""
    loads2d = np.atleast_2d(np.asarray(loads2d, dtype=np.float64))
    if loads2d.shape[0] != len(loads2d) or loads2d.ndim == 1:
        loads2d = loads2d.T
    n = loads2d.shape[0]
    target = np.maximum(loads2d.sum(0) / n_bins, 1e-9)
    order = np.argsort(-loads2d.sum(1), kind="stable")
    fill = np.zeros(n_bins, dtype=np.int64)
    bload = np.zeros((n_bins, loads2d.shape[1]))
    newid = np.empty(n, dtype=np.int64)
    for it in order:
        score = ((bload + loads2d[it]) / target).max(axis=1)
        score[fill >= cap] = np.inf
        b = int(np.argmin(score))
        newid[it] = b * cap + fill[b]
        fill[b] += 1
        bload[b] += loads2d[it]
    return newid


def preprocess(X, vertex, edges, X0, W1, b1, W2, b2, W3, b3,
               M=25000, ncores=8, G=8, lo_split=32768):
    """Build per-core input maps + compile-time metadata."""
    X = np.asarray(X, dtype=np.float32)
    X0 = np.asarray(X0, dtype=np.float32)
    vertex = np.asarray(vertex).astype(np.int64)
    edges = np.asarray(edges).astype(np.int64)
    W1 = np.asarray(W1, dtype=np.float32)
    b1 = np.asarray(b1, dtype=np.float32)
    W2 = np.asarray(W2, dtype=np.float32)
    b2 = np.asarray(b2, dtype=np.float32)
    W3 = np.asarray(W3, dtype=np.float32)
    b3 = np.asarray(b3, dtype=np.float32)

    N, Din = X.shape
    NNZ = len(vertex)
    Dout = W3.shape[1]
    assert Din == D and Dout == D

    LO = min(lo_split, N)
    EPC = M // ncores
    VPC = N // ncores
    assert M % ncores == 0 and N % ncores == 0
    NW2 = (EPC + P - 1) // P
    NW3 = (VPC + P - 1) // P

    alpha = 0.5
    W2a = W2[:D]
    W2b = W2[D:]
    deg = np.bincount(vertex, minlength=N).astype(np.float64)
    edeg = np.bincount(edges, minlength=M).astype(np.float64)
    wdeg = np.bincount(vertex, weights=edeg[edges], minlength=N)

    Wa = ((1.0 - alpha) * W2a).astype(np.float32)
    Wt = ((1.0 - alpha) * (W1.astype(np.float64) @ W2b.astype(np.float64))).astype(np.float32)
    b1w = (W2b.astype(np.float64).T @ b1.astype(np.float64))  # b1 @ W2b
    b3_full = np.tile(b3[None, :], (P, 1)).astype(np.float32)

    core_edge = edges // EPC
    core_vert = vertex // VPC

    EPCP = NW2 * P  # padded (relabeled) local edge id space
    VPCP = NW3 * P  # padded (relabeled) local vertex id space
    H = (EPCP // 2 // P) * P if EPCP >= 2 * P else max(EPCP // 2, 1)

    # AllGather piece boundaries over the padded local edge space
    HB = [0, H, EPCP] if EPCP >= 2 * P else [0, EPCP]
    NP3 = len(HB) - 1

    # per-core relabeling so every 128-wide window has a balanced pair count
    # in EVERY gather stream (lo/hi table halves, AllGather pieces), not just
    # in total -- the cross-core max of per-stream window counts sets padding.
    mlo_all = vertex < LO
    eperm = []
    for i in range(ncores):
        sel = core_edge == i
        pe_raw = edges[sel] - i * EPC
        ml = mlo_all[sel]
        lo_e = np.bincount(pe_raw[ml], minlength=EPC)
        hi_e = np.bincount(pe_raw[~ml], minlength=EPC)
        eperm.append(_balance_bins(np.stack([lo_e, hi_e], 1), NW2, P))
    eperm_all = np.concatenate(eperm)  # indexed by global edge id

    el_all = eperm_all[edges]  # relabeled local edge id of every pair
    vperm = []
    for i in range(ncores):
        sel = core_vert == i
        pv_raw = vertex[sel] - i * VPC
        el_i = el_all[sel]
        piece_loads = [
            np.bincount(pv_raw[(el_i >= HB[k]) & (el_i < HB[k + 1])], minlength=VPC)
            for k in range(NP3)
        ]
        vperm.append(_balance_bins(np.stack(piece_loads, 1), NW3, P))

    # ---- per-core pair lists
    s2lo_w, s2hi_w = [], []
    s2lo_pairs, s2hi_pairs = [], []
    s3_w = [[] for _ in range(NP3)]
    s3_pairs = [[] for _ in range(NP3)]
    for i in range(ncores):
        sel = np.nonzero(core_edge == i)[0]
        pv = vertex[sel]
        pe = eperm[i][edges[sel] - i * EPC]
        mlo = pv < LO
        for store_w, store_p, v, e in (
            (s2lo_w, s2lo_pairs, pv[mlo], pe[mlo]),
            (s2hi_w, s2hi_pairs, pv[~mlo] - LO, pe[~mlo]),
        ):
            store_w.append(np.bincount(e // P, minlength=NW2))
            store_p.append((e, v))
        sel = np.nonzero(core_vert == i)[0]
        pe = edges[sel]
        pvl = vperm[i][vertex[sel] - i * VPC]
        el = eperm_all[pe]
        er = pe // EPC
        # split by AllGather piece; gather idx into each piece's layout
        for k in range(NP3):
            mk = (el >= HB[k]) & (el < HB[k + 1])
            ik = er[mk] * (HB[k + 1] - HB[k]) + (el[mk] - HB[k])
            s3_w[k].append(np.bincount(pvl[mk] // P, minlength=NW3))
            s3_pairs[k].append((pvl[mk], ik))

    C2lo = _chunk_counts(s2lo_w, NW2, G)
    C2hi = _chunk_counts(s2hi_w, NW2, G)
    C3 = [_chunk_counts(s3_w[k], NW3, G, min_one=(k == 0)) for k in range(NP3)]
    S2LO, S2HI = int(C2lo.sum()), int(C2hi.sum())
    S3 = [int(c.sum()) for c in C3]

    iota = np.tile(np.arange(P, dtype=np.float32), G)[None, :].repeat(P, axis=0)
    iota = np.ascontiguousarray(iota)

    import ml_dtypes
    X_bf16 = X.astype(ml_dtypes.bfloat16)
    iota = iota.astype(ml_dtypes.bfloat16)
    W3h = W3.astype(ml_dtypes.bfloat16)

    in_maps = []
    for i in range(ncores):
        e, v = s2lo_pairs[i]
        lo_idx, lo_seg = _pack_stream(e, v, NW2, C2lo)
        e, v = s2hi_pairs[i]
        hi_idx, hi_seg = _pack_stream(e, v, NW2, C2hi)
        s3_packed = []
        for k in range(NP3):
            pvl, ik = s3_pairs[k][i]
            s3_packed.append(_pack_stream(pvl, ik, NW3, C3[k]))

        sl = slice(i * VPC, (i + 1) * VPC)
        xd_full = np.zeros((VPCP, D), dtype=np.float64)
        xd_full[vperm[i]] = X[sl].astype(np.float64) * deg[sl, None]
        xd_t = np.ascontiguousarray(xd_full.T).astype(np.float32)
        x0h = alpha * X0[sl].astype(np.float64).T \
            + (1.0 - alpha) * (np.outer(b2, deg[sl]) + np.outer(b1w, wdeg[sl]))
        x0h_full = np.zeros((D, VPCP), dtype=np.float64)
        x0h_full[:, vperm[i]] = x0h
        x0h_t = np.ascontiguousarray(x0h_full).astype(np.float32)

        im = {
            "x_tab": X_bf16,
            "s2lo_idx": _wrap_idx(lo_idx, G), "s2lo_seg": _seg_tile(lo_seg),
            "s2hi_idx": _wrap_idx(hi_idx, G), "s2hi_seg": _seg_tile(hi_seg),
            "iota": iota,
            "xd_t": xd_t,
            "x0h_t": x0h_t,
            "wa": Wa, "wt": Wt, "w3": W3h, "b3f": b3_full,
        }
        for k in range(NP3):
            pidx, pseg = s3_packed[k]
            im[f"s3p{k}_idx"] = _wrap_idx(pidx, G)
            im[f"s3p{k}_seg"] = _seg_tile(pseg)
        in_maps.append(im)

    meta = dict(N=N, M=M, NNZ=NNZ, ncores=ncores, G=G, LO=LO, H=H,
                EPC=EPC, VPC=VPC, NW2=NW2, NW3=NW3, EPCP=EPCP, VPCP=VPCP,
                HB=HB, NP3=NP3,
                C2lo=C2lo.tolist(), C2hi=C2hi.tolist(),
                C3=[c.tolist() for c in C3],
                S2LO=S2LO, S2HI=S2HI, S3=S3)
    meta["vperm"] = vperm
    return in_maps, meta


# ---------------------------------------------------------------------------
# device program
# ---------------------------------------------------------------------------

def build_program(meta):
    import concourse.bacc as bacc
    import concourse.bass as bass  # noqa: F401
    import concourse.mybir as mybir
    import concourse.tile as tile
    from concourse._compat import get_trn_type
    from concourse import library_config
    from concourse.tile_rust import add_dep_helper

    f32 = mybir.dt.float32
    bf16 = mybir.dt.bfloat16
    i16 = mybir.dt.int16

    ncores = meta["ncores"]
    G = meta["G"]
    N, M = meta["N"], meta["M"]
    LO = meta["LO"]
    EPC, VPC = meta["EPCP"], meta["VPCP"]
    NW2, NW3 = meta["NW2"], meta["NW3"]
    C2lo, C2hi = meta["C2lo"], meta["C2hi"]
    C3 = meta["C3"]
    S2LO, S2HI = meta["S2LO"], meta["S2HI"]
    S3 = meta["S3"]
    HB, NP3 = meta["HB"], meta["NP3"]
    H = meta["H"]
    GP = G * P

    nc = bacc.Bacc(get_trn_type() or "TRN2", num_devices=ncores, num_swdge_queues=4,
                   dynamic_dma_scratch_size=32768)

    x_tab = nc.declare_dram_parameter("x_tab", [N, D], bf16, isOutput=False)
    s2lo_idx = nc.declare_dram_parameter("s2lo_idx", [P, S2LO * 8], i16, isOutput=False)
    s2lo_seg = nc.declare_dram_parameter("s2lo_seg", [P, S2LO], bf16, isOutput=False)
    s2hi_idx = nc.declare_dram_parameter("s2hi_idx", [P, S2HI * 8], i16, isOutput=False)
    s2hi_seg = nc.declare_dram_parameter("s2hi_seg", [P, S2HI], bf16, isOutput=False)
    s3_idx_d, s3_seg_d = [], []
    for k in range(NP3):
        s3_idx_d.append(nc.declare_dram_parameter(f"s3p{k}_idx", [P, S3[k] * 8], i16, isOutput=False))
        s3_seg_d.append(nc.declare_dram_parameter(f"s3p{k}_seg", [P, S3[k]], bf16, isOutput=False))
    iota_d = nc.declare_dram_parameter("iota", [P, GP], bf16, isOutput=False)
    xd_d = nc.declare_dram_parameter("xd_t", [D, VPC], f32, isOutput=False)
    x0h_d = nc.declare_dram_parameter("x0h_t", [D, VPC], f32, isOutput=False)
    wa_d = nc.declare_dram_parameter("wa", [D, D], f32, isOutput=False)
    wt_d = nc.declare_dram_parameter("wt", [D, D], f32, isOutput=False)
    w3_d = nc.declare_dram_parameter("w3", [D, D], bf16, isOutput=False)
    b3f_d = nc.declare_dram_parameter("b3f", [P, D], f32, isOutput=False)
    out_d = nc.declare_dram_parameter("out", [VPC, D], f32, isOutput=True)

    se_slice = nc.dram_tensor("se_slice", [EPC, D], bf16)
    se_p = [nc.dram_tensor(f"se_p{k}", [ncores * (HB[k + 1] - HB[k]), D], bf16,
                           addr_space="Shared") for k in range(NP3)]

    with tile.TileContext(nc) as tc:
        with (
            tc.tile_pool(name="consts", bufs=1) as consts,
            tc.tile_pool(name="resident", bufs=1) as resident,
            tc.tile_pool(name="gat", bufs=10) as gat,
            tc.tile_pool(name="ohp", bufs=10) as ohp,
            tc.tile_pool(name="sep", bufs=3) as sep,
            tc.tile_pool(name="winp", bufs=5, space="PSUM") as winp,
            tc.tile_pool(name="zvp", bufs=1, space="PSUM") as zvp,
            tc.tile_pool(name="outp", bufs=2, space="PSUM") as outp,
        ):
            # ---- resident loads
            iota_t = consts.tile([P, G, P], bf16)
            nc.sync.dma_start(iota_t[:], iota_d[:].rearrange("p (g q) -> p g q", q=P))
            wa_t = consts.tile([D, D], f32)
            nc.sync.dma_start(wa_t[:], wa_d[:])
            wt_t = consts.tile([D, D], f32)
            nc.sync.dma_start(wt_t[:], wt_d[:])
            w3_t = consts.tile([D, D], bf16)
            nc.sync.dma_start(w3_t[:], w3_d[:])
            b3f_t = consts.tile([P, D], f32)
            nc.sync.dma_start(b3f_t[:], b3f_d[:])

            nc.gpsimd.load_library(library_config.mlp)
            npairs_reg = nc.gpsimd.to_reg(GP)
            qctr = [0]

            class Stream:
                def __init__(self, name, idx_d, seg_d, n_chunks, table_ap, counts,
                             split_head=False):
                    self.name = name
                    self.counts = counts
                    self.off = np.concatenate([[0], np.cumsum(counts)[:-1]]).astype(int)
                    self.table_ap = table_ap
                    self.idx_t = resident.tile([P, n_chunks * 8], i16, tag=f"idx_{name}")
                    nc.sync.dma_start(self.idx_t[:], idx_d[:])
                    self.seg_t = resident.tile([P, n_chunks], bf16, tag=f"seg_{name}")
                    nc.sync.dma_start(self.seg_t[:], seg_d[:])
                    self.batches = {}
                    self.gather_insts = []

                def batch(self, b):
                    if b not in self.batches:
                        gt = gat.tile([P, G, D], bf16, tag="gat")
                        inst = nc.gpsimd.dma_gather(
                            gt[:],
                            self.table_ap,
                            self.idx_t[:, b * G * 8:(b + 1) * G * 8],
                            GP,
                            npairs_reg,
                            D,
                            queue_num=qctr[0] % 4,
                        )
                        qctr[0] += 1
                        self.gather_insts.append(inst)
                        oh = ohp.tile([P, G, P], bf16, tag="oh")
                        nc.vector.tensor_tensor(
                            out=oh[:],
                            in0=iota_t[:],
                            in1=self.seg_t[:, b * G:(b + 1) * G].broadcast_to([P, G, P]),
                            op=mybir.AluOpType.is_equal,
                        )
                        self.batches[b] = (gt, oh)
                    return self.batches[b]

            lo = Stream("s2lo", s2lo_idx, s2lo_seg, S2LO, x_tab[0:LO, :], C2lo,
                        split_head=True)
            streams2 = [lo]
            if LO < N:
                hi = Stream("s2hi", s2hi_idx, s2hi_seg, S2HI, x_tab[LO:N, :], C2hi,
                            split_head=True)
                streams2.append(hi)

            # ---- stage A: Se[e] = sum_{pairs with edge e} X[v]
            # AllGather fires per piece as its windows finish flushing.
            flushes = [[] for _ in range(NP3)]
            ags = [None] * NP3
            w_ag = [HB[k + 1] // P - 1 for k in range(NP3)]
            for w in range(NW2):
                total_k = sum(int(s.counts[w]) for s in streams2)
                psum_w = winp.tile([P, P], f32, tag="win")
                k = 0
                for s in streams2:
                    for c in range(int(s.off[w]), int(s.off[w]) + int(s.counts[w])):
                        b, cl = divmod(c, G)
                        gt, oh = s.batch(b)
                        nc.tensor.matmul(
                            psum_w[:],
                            lhsT=oh[:, cl, :],
                            rhs=gt[:, cl, :],
                            start=(k == 0),
                            stop=(k == total_k - 1),
                        )
                        k += 1
                st = sep.tile([P, P], bf16, tag="seflush")
                nc.vector.tensor_copy(out=st[:], in_=psum_w[:])
                fl = nc.sync.dma_start(out=se_slice[w * P:(w + 1) * P, :], in_=st[:])
                kp = next(k2 for k2 in range(NP3) if w * P < HB[k2 + 1])
                flushes[kp].append(fl)
                if w == w_ag[kp]:
                    ags[kp] = nc.gpsimd.collective_compute(
                        "AllGather", mybir.AluOpType.bypass,
                        replica_groups=[list(range(ncores))],
                        ins=[se_slice[HB[kp]:HB[kp + 1], :]], outs=[se_p[kp][:]])
                    for f in flushes[kp]:
                        add_dep_helper(ags[kp].ins, f.ins,
                                       reason=f"AG{kp} reads its se_slice piece")

            # ---- stage B: T[v] = sum_{pairs with vertex v} Se[e]
            # one pass per AllGather piece; the dense tail (stages C/D) is
            # interleaved as windows finalize during the last pass.
            streams3 = [Stream(f"s3p{k}", s3_idx_d[k], s3_seg_d[k], S3[k],
                               se_p[k][:], C3[k]) for k in range(NP3)]
            Tt = resident.tile([P, NW3 * P], f32, tag="Tt")
            xd_t = resident.tile([D, VPC], f32, tag="xd")
            nc.sync.dma_start(xd_t[:], xd_d[:])
            x0h_t = resident.tile([D, VPC], f32, tag="x0h")
            nc.sync.dma_start(x0h_t[:], x0h_d[:])
            zt_t = resident.tile([D, VPC], bf16, tag="zt")

            RT = 512

            def emit_c_tile(rt):
                s0 = rt * RT
                L = min(RT, VPC - s0)
                pz = zvp.tile([P, RT], f32, tag="zv")
                nc.tensor.matmul(pz[:, :L], lhsT=wa_t[:], rhs=xd_t[:, s0:s0 + L],
                                 start=True, stop=False)
                nc.tensor.matmul(pz[:, :L], lhsT=wt_t[:], rhs=Tt[:, s0:s0 + L],
                                 start=False, stop=True)
                nc.vector.tensor_add(out=zt_t[:, s0:s0 + L], in0=pz[:, :L],
                                     in1=x0h_t[:, s0:s0 + L])
                for ot in range(s0 // P, (s0 + L + P - 1) // P):
                    o0 = ot * P
                    Lo = min(P, VPC - o0)
                    po = outp.tile([P, P], f32, tag="out")
                    nc.tensor.matmul(po[:Lo, :], lhsT=zt_t[:, o0:o0 + Lo], rhs=w3_t[:],
                                     start=True, stop=True)
                    st = sep.tile([P, P], f32, tag="outflush")
                    nc.vector.tensor_tensor(out=st[:Lo, :], in0=po[:Lo, :],
                                            in1=b3f_t[:Lo, :], op=mybir.AluOpType.add)
                    nc.sync.dma_start(out=out_d[o0:o0 + Lo, :], in_=st[:Lo, :])

            n_ctiles = (VPC + RT - 1) // RT
            for kp, s3 in enumerate(streams3):
                last = kp == NP3 - 1
                done_c = 0
                for w in range(NW3):
                    total_k = int(s3.counts[w])
                    if total_k > 0:
                        psum_w = winp.tile([P, P], f32, tag="win")
                        for k, c in enumerate(range(int(s3.off[w]), int(s3.off[w]) + total_k)):
                            b, cl = divmod(c, G)
                            gt, oh = s3.batch(b)
                            nc.tensor.matmul(
                                psum_w[:],
                                lhsT=gt[:, cl, :],
                                rhs=oh[:, cl, :],
                                start=(k == 0),
                                stop=(k == total_k - 1),
                            )
                        if kp == 0:
                            nc.vector.tensor_copy(out=Tt[:, w * P:(w + 1) * P], in_=psum_w[:])
                        else:
                            nc.vector.tensor_add(out=Tt[:, w * P:(w + 1) * P],
                                                 in0=Tt[:, w * P:(w + 1) * P], in1=psum_w[:])
                    # emit any C tiles fully covered by finalized windows
                    if last:
                        while done_c < n_ctiles and (done_c + 1) * RT <= (w + 1) * P:
                            emit_c_tile(done_c)
                            done_c += 1
                if last:
                    while done_c < n_ctiles:
                        emit_c_tile(done_c)
                        done_c += 1

            for kp, s3 in enumerate(streams3):
                for inst in s3.gather_insts:
                    add_dep_helper(inst.ins, ags[kp].ins,
                                   reason=f"pass-{kp} gathers read se_p{kp}")

    return nc


# ---------------------------------------------------------------------------
# entry point
# ---------------------------------------------------------------------------

def _run(inputs, trace=False, M=25000, ncores=8, G=8, lo_split=32768):
    import sys
    if "/opt/trn_rl_repo" not in sys.path:
        sys.path.insert(0, "/opt/trn_rl_repo")
    from concourse.bass_utils import run_bass_kernel_spmd

    in_maps, meta = preprocess(**inputs, M=M, ncores=ncores, G=G, lo_split=lo_split)
    nc = build_program(meta)
    if not nc.is_finalized():
        nc.finalize()
    res = run_bass_kernel_spmd(nc, in_maps, list(range(ncores)), trace=trace)
    vperm = meta["vperm"]
    out = np.concatenate(
        [np.asarray(res.results[i]["out"])[vperm[i]] for i in range(ncores)], axis=0)
    return out, res


def kernel(**inputs):
    out, _ = _run(inputs)
    return out



# revision 29
# speedup vs baseline: 1.1542x; 1.0135x over previous
"""Trainium2 Bass kernel for nn_EquivSetConv (hypergraph message passing).

Reference computation:
    Xve = (X @ W1 + b1)[vertex]
    Xe  = segment_sum(Xve, edges, M)
    Xev = Xe[edges]
    H   = concat([X[vertex], Xev], -1) @ W2 + b2
    Xv  = segment_sum(H, vertex, N)
    out = ((1-a)*Xv + a*X0) @ W3 + b3

Algebraic restructure (A[v,e] = #incidence pairs (v,e)):
    Se  = A^T @ X                          (segmented sum of raw X rows per edge)
    Xe  = Se @ W1 + edeg x b1
    T   = A @ Se                           (segmented sum of Se rows per vertex)
    Xv  = deg . (X @ W2a) + T @ (W1 @ W2b) + deg x b2 + wdeg x (b1 @ W2b)
    out = ((1-a)Xv + a X0) @ W3 + b3

So the 800k-row dense matmul disappears; the kernel is two sparse
gather+segmented-sum stages plus small dense matmuls.

Sharding over 8 cores: stage A partitioned by edge range (each core owns
M/8 edges and all pairs incident to them -> computes its Se slice fully,
no cross-core reduction), one AllGather of Se, stage B partitioned by
vertex range (each core owns N/8 vertices -> computes its output rows
end to end). The only collective is the 1.6MB/rank AllGather.

Sparse stages on device: host sorts pairs by destination segment and
packs them into 128-pair chunks that are pure in a 128-wide segment
window.  For each chunk: dma_gather 128 source rows (pair p -> SBUF
partition p), build a one-hot [pair, segment-slot] matrix on DVE
(iota == seg), and accumulate with one PE matmul into the window's PSUM
tile.  Windows flush to SBUF/DRAM when complete.
"""

import numpy as np

P = 128
D = 128


# ---------------------------------------------------------------------------
# host-side preprocessing
# ---------------------------------------------------------------------------

def _pack_stream(seg_local, gidx, n_windows, chunk_counts):
    """Pack pairs (sorted by window) into window-pure 128-slot chunks.

    seg_local: [n] int, segment id LOCAL to the stream's window grid
               (seg_local // 128 = window, seg_local % 128 = slot)
    gidx:      [n] int, gather index of each pair
    chunk_counts: [n_windows] int, chunks allocated per window (shared
               across all cores so the program structure is identical).

    Returns (idx16, segf) flat arrays of length sum(chunk_counts)*128,
    pad slots have idx 0 / seg -1.
    """
    total_chunks = int(np.sum(chunk_counts))
    tot = total_chunks * P
    idx16 = np.zeros(tot, dtype=np.int16)
    segf = np.full(tot, -1.0, dtype=np.float32)
    if len(seg_local) == 0:
        return idx16, segf

    order = np.argsort(seg_local, kind="stable")
    seg_s = seg_local[order]
    gidx_s = gidx[order]
    win = seg_s // P

    # position of each pair: chunk_base[win]*128 + rank-within-window
    chunk_base = np.concatenate([[0], np.cumsum(chunk_counts)[:-1]])
    win_start = np.searchsorted(win, np.arange(n_windows), side="left")
    rank = np.arange(len(win)) - win_start[win]
    pos = chunk_base[win] * P + rank
    idx16[pos] = gidx_s.astype(np.int16)
    segf[pos] = (seg_s % P).astype(np.float32)
    return idx16, segf


def _wrap_idx(idx16, G):
    """Reshape a flat per-stream idx array into the dma_gather SBUF layout.

    Within each batch of G*128 indices, index i lives at
    [partition i%16, column i//16]; batches are side by side (the last
    batch may cover fewer than G chunks; it is zero-padded for layout).
    Output [128, ceil(total_chunks/G)*G*8] int16 (rows 0..15 replicated).
    """
    rem = (-len(idx16)) % (G * P)
    if rem:
        idx16 = np.concatenate([idx16, np.zeros(rem, dtype=np.int16)])
    nb = len(idx16) // (G * P)
    blocks = [idx16[b * G * P:(b + 1) * G * P].reshape(G * 8, 16).T for b in range(nb)]
    arr16 = np.hstack(blocks)  # [16, nb*G*8]
    return np.tile(arr16, (8, 1)).astype(np.int16)


def _seg_tile(segf):
    """[total_chunks*128] -> [128, total_chunks]: pair (chunk c, part p)."""
    import ml_dtypes
    n_chunks = len(segf) // P
    return np.ascontiguousarray(segf.reshape(n_chunks, P).T).astype(ml_dtypes.bfloat16)


def _chunk_counts(windows_per_core, n_windows, G, min_one=True):
    """windows_per_core: list over cores of [n_windows] pair counts.
    Returns per-window chunk counts (max over cores), padded so the
    total is a multiple of G."""
    counts = np.zeros(n_windows, dtype=np.int64)
    for wc in windows_per_core:
        counts = np.maximum(counts, (wc + P - 1) // P)
    if min_one:
        counts = np.maximum(counts, 1)
    tot = int(counts.sum())
    counts[-1] += (-tot) % G
    return counts


def _balance_bins(loads2d, n_bins, cap):
    """Greedy multi-dim LPT: assign items (rows of loads2d, one load per
    stream dimension) to n_bins bins of capacity `cap` items, keeping every
    per-dimension bin load balanced.  Returns [n] -> bin*cap + slot."""
    loads2d = np.atleast_2d(np.asarray(loads2d, dtype=np.float64))
    if loads2d.shape[0] != len(loads2d) or loads2d.ndim == 1:
        loads2d = loads2d.T
    n = loads2d.shape[0]
    target = np.maximum(loads2d.sum(0) / n_bins, 1e-9)
    order = np.argsort(-loads2d.sum(1), kind="stable")
    fill = np.zeros(n_bins, dtype=np.int64)
    bload = np.zeros((n_bins, loads2d.shape[1]))
    newid = np.empty(n, dtype=np.int64)
    for it in order:
        score = ((bload + loads2d[it]) / target).max(axis=1)
        score[fill >= cap] = np.inf
        b = int(np.argmin(score))
        newid[it] = b * cap + fill[b]
        fill[b] += 1
        bload[b] += loads2d[it]
    return newid


def preprocess(X, vertex, edges, X0, W1, b1, W2, b2, W3, b3,
               M=25000, ncores=8, G=8, lo_split=32768):
    """Build per-core input maps + compile-time metadata."""
    X = np.asarray(X, dtype=np.float32)
    X0 = np.asarray(X0, dtype=np.float32)
    vertex = np.asarray(vertex).astype(np.int64)
    edges = np.asarray(edges).astype(np.int64)
    W1 = np.asarray(W1, dtype=np.float32)
    b1 = np.asarray(b1, dtype=np.float32)
    W2 = np.asarray(W2, dtype=np.float32)
    b2 = np.asarray(b2, dtype=np.float32)
    W3 = np.asarray(W3, dtype=np.float32)
    b3 = np.asarray(b3, dtype=np.float32)

    N, Din = X.shape
    NNZ = len(vertex)
    Dout = W3.shape[1]
    assert Din == D and Dout == D

    LO = min(lo_split, N)
    EPC = M // ncores
    VPC = N // ncores
    assert M % ncores == 0 and N % ncores == 0
    NW2 = (EPC + P - 1) // P
    NW3 = (VPC + P - 1) // P

    alpha = 0.5
    W2a = W2[:D]
    W2b = W2[D:]
    deg = np.bincount(vertex, minlength=N).astype(np.float64)
    edeg = np.bincount(edges, minlength=M).astype(np.float64)
    wdeg = np.bincount(vertex, weights=edeg[edges], minlength=N)

    Wa = ((1.0 - alpha) * W2a).astype(np.float32)
    Wt = ((1.0 - alpha) * (W1.astype(np.float64) @ W2b.astype(np.float64))).astype(np.float32)
    b1w = (W2b.astype(np.float64).T @ b1.astype(np.float64))  # b1 @ W2b
    b3_full = np.tile(b3[None, :], (P, 1)).astype(np.float32)

    core_edge = edges // EPC
    core_vert = vertex // VPC

    EPCP = NW2 * P  # padded (relabeled) local edge id space
    VPCP = NW3 * P  # padded (relabeled) local vertex id space
    H = (EPCP // 2 // P) * P if EPCP >= 2 * P else max(EPCP // 2, 1)

    # AllGather piece boundaries over the padded local edge space
    HB = [0, H, EPCP] if EPCP >= 2 * P else [0, EPCP]
    NP3 = len(HB) - 1

    # per-core relabeling so every 128-wide window has a balanced pair count
    # in EVERY gather stream (lo/hi table halves, AllGather pieces), not just
    # in total -- the cross-core max of per-stream window counts sets padding.
    mlo_all = vertex < LO
    eperm = []
    for i in range(ncores):
        sel = core_edge == i
        pe_raw = edges[sel] - i * EPC
        ml = mlo_all[sel]
        lo_e = np.bincount(pe_raw[ml], minlength=EPC)
        hi_e = np.bincount(pe_raw[~ml], minlength=EPC)
        eperm.append(_balance_bins(np.stack([lo_e, hi_e], 1), NW2, P))
    eperm_all = np.concatenate(eperm)  # indexed by global edge id

    el_all = eperm_all[edges]  # relabeled local edge id of every pair
    vperm = []
    for i in range(ncores):
        sel = core_vert == i
        pv_raw = vertex[sel] - i * VPC
        el_i = el_all[sel]
        piece_loads = [
            np.bincount(pv_raw[(el_i >= HB[k]) & (el_i < HB[k + 1])], minlength=VPC)
            for k in range(NP3)
        ]
        vperm.append(_balance_bins(np.stack(piece_loads, 1), NW3, P))

    # ---- per-core pair lists
    s2lo_w, s2hi_w = [], []
    s2lo_pairs, s2hi_pairs = [], []
    s3_w = [[] for _ in range(NP3)]
    s3_pairs = [[] for _ in range(NP3)]
    for i in range(ncores):
        sel = np.nonzero(core_edge == i)[0]
        pv = vertex[sel]
        pe = eperm[i][edges[sel] - i * EPC]
        mlo = pv < LO
        for store_w, store_p, v, e in (
            (s2lo_w, s2lo_pairs, pv[mlo], pe[mlo]),
            (s2hi_w, s2hi_pairs, pv[~mlo] - LO, pe[~mlo]),
        ):
            store_w.append(np.bincount(e // P, minlength=NW2))
            store_p.append((e, v))
        sel = np.nonzero(core_vert == i)[0]
        pe = edges[sel]
        pvl = vperm[i][vertex[sel] - i * VPC]
        el = eperm_all[pe]
        er = pe // EPC
        # split by AllGather piece; gather idx into each piece's layout
        for k in range(NP3):
            mk = (el >= HB[k]) & (el < HB[k + 1])
            ik = er[mk] * (HB[k + 1] - HB[k]) + (el[mk] - HB[k])
            s3_w[k].append(np.bincount(pvl[mk] // P, minlength=NW3))
            s3_pairs[k].append((pvl[mk], ik))

    C2lo = _chunk_counts(s2lo_w, NW2, G)
    C2hi = _chunk_counts(s2hi_w, NW2, G)
    C3 = [_chunk_counts(s3_w[k], NW3, G, min_one=(k == 0)) for k in range(NP3)]
    S2LO, S2HI = int(C2lo.sum()), int(C2hi.sum())
    S3 = [int(c.sum()) for c in C3]

    iota = np.tile(np.arange(P, dtype=np.float32), G)[None, :].repeat(P, axis=0)
    iota = np.ascontiguousarray(iota)

    import ml_dtypes
    X_bf16 = X.astype(ml_dtypes.bfloat16)
    iota = iota.astype(ml_dtypes.bfloat16)
    W3h = W3.astype(ml_dtypes.bfloat16)

    in_maps = []
    for i in range(ncores):
        e, v = s2lo_pairs[i]
        lo_idx, lo_seg = _pack_stream(e, v, NW2, C2lo)
        e, v = s2hi_pairs[i]
        hi_idx, hi_seg = _pack_stream(e, v, NW2, C2hi)
        s3_packed = []
        for k in range(NP3):
            pvl, ik = s3_pairs[k][i]
            s3_packed.append(_pack_stream(pvl, ik, NW3, C3[k]))

        sl = slice(i * VPC, (i + 1) * VPC)
        xd_full = np.zeros((VPCP, D), dtype=np.float64)
        xd_full[vperm[i]] = X[sl].astype(np.float64) * deg[sl, None]
        xd_t = np.ascontiguousarray(xd_full.T).astype(np.float32)
        x0h = alpha * X0[sl].astype(np.float64).T \
            + (1.0 - alpha) * (np.outer(b2, deg[sl]) + np.outer(b1w, wdeg[sl]))
        x0h_full = np.zeros((D, VPCP), dtype=np.float64)
        x0h_full[:, vperm[i]] = x0h
        x0h_t = np.ascontiguousarray(x0h_full).astype(np.float32)

        im = {
            "x_tab": X_bf16,
            "s2lo_idx": _wrap_idx(lo_idx, G), "s2lo_seg": _seg_tile(lo_seg),
            "s2hi_idx": _wrap_idx(hi_idx, G), "s2hi_seg": _seg_tile(hi_seg),
            "iota": iota,
            "xd_t": xd_t,
            "x0h_t": x0h_t,
            "wa": Wa, "wt": Wt, "w3": W3h, "b3f": b3_full,
        }
        for k in range(NP3):
            pidx, pseg = s3_packed[k]
            im[f"s3p{k}_idx"] = _wrap_idx(pidx, G)
            im[f"s3p{k}_seg"] = _seg_tile(pseg)
        in_maps.append(im)

    meta = dict(N=N, M=M, NNZ=NNZ, ncores=ncores, G=G, LO=LO, H=H,
                EPC=EPC, VPC=VPC, NW2=NW2, NW3=NW3, EPCP=EPCP, VPCP=VPCP,
                HB=HB, NP3=NP3,
                C2lo=C2lo.tolist(), C2hi=C2hi.tolist(),
                C3=[c.tolist() for c in C3],
                S2LO=S2LO, S2HI=S2HI, S3=S3)
    meta["vperm"] = vperm
    return in_maps, meta


# ---------------------------------------------------------------------------
# device program
# ---------------------------------------------------------------------------

def build_program(meta):
    import concourse.bacc as bacc
    import concourse.bass as bass  # noqa: F401
    import concourse.mybir as mybir
    import concourse.tile as tile
    from concourse._compat import get_trn_type
    from concourse import library_config
    from concourse.tile_rust import add_dep_helper

    f32 = mybir.dt.float32
    bf16 = mybir.dt.bfloat16
    i16 = mybir.dt.int16

    ncores = meta["ncores"]
    G = meta["G"]
    N, M = meta["N"], meta["M"]
    LO = meta["LO"]
    EPC, VPC = meta["EPCP"], meta["VPCP"]
    NW2, NW3 = meta["NW2"], meta["NW3"]
    C2lo, C2hi = meta["C2lo"], meta["C2hi"]
    C3 = meta["C3"]
    S2LO, S2HI = meta["S2LO"], meta["S2HI"]
    S3 = meta["S3"]
    HB, NP3 = meta["HB"], meta["NP3"]
    H = meta["H"]
    GP = G * P

    nc = bacc.Bacc(get_trn_type() or "TRN2", num_devices=ncores, num_swdge_queues=4,
                   dynamic_dma_scratch_size=32768)

    x_tab = nc.declare_dram_parameter("x_tab", [N, D], bf16, isOutput=False)
    s2lo_idx = nc.declare_dram_parameter("s2lo_idx", [P, S2LO * 8], i16, isOutput=False)
    s2lo_seg = nc.declare_dram_parameter("s2lo_seg", [P, S2LO], bf16, isOutput=False)
    s2hi_idx = nc.declare_dram_parameter("s2hi_idx", [P, S2HI * 8], i16, isOutput=False)
    s2hi_seg = nc.declare_dram_parameter("s2hi_seg", [P, S2HI], bf16, isOutput=False)
    s3_idx_d, s3_seg_d = [], []
    for k in range(NP3):
        s3_idx_d.append(nc.declare_dram_parameter(f"s3p{k}_idx", [P, S3[k] * 8], i16, isOutput=False))
        s3_seg_d.append(nc.declare_dram_parameter(f"s3p{k}_seg", [P, S3[k]], bf16, isOutput=False))
    iota_d = nc.declare_dram_parameter("iota", [P, GP], bf16, isOutput=False)
    xd_d = nc.declare_dram_parameter("xd_t", [D, VPC], f32, isOutput=False)
    x0h_d = nc.declare_dram_parameter("x0h_t", [D, VPC], f32, isOutput=False)
    wa_d = nc.declare_dram_parameter("wa", [D, D], f32, isOutput=False)
    wt_d = nc.declare_dram_parameter("wt", [D, D], f32, isOutput=False)
    w3_d = nc.declare_dram_parameter("w3", [D, D], bf16, isOutput=False)
    b3f_d = nc.declare_dram_parameter("b3f", [P, D], f32, isOutput=False)
    out_d = nc.declare_dram_parameter("out", [VPC, D], f32, isOutput=True)

    se_slice = nc.dram_tensor("se_slice", [EPC, D], bf16)
    se_p = [nc.dram_tensor(f"se_p{k}", [ncores * (HB[k + 1] - HB[k]), D], bf16,
                           addr_space="Shared") for k in range(NP3)]

    with tile.TileContext(nc) as tc:
        with (
            tc.tile_pool(name="consts", bufs=1) as consts,
            tc.tile_pool(name="resident", bufs=1) as resident,
            tc.tile_pool(name="gat", bufs=10) as gat,
            tc.tile_pool(name="ohp", bufs=10) as ohp,
            tc.tile_pool(name="sep", bufs=3) as sep,
            tc.tile_pool(name="winp", bufs=5, space="PSUM") as winp,
            tc.tile_pool(name="zvp", bufs=1, space="PSUM") as zvp,
            tc.tile_pool(name="outp", bufs=2, space="PSUM") as outp,
        ):
            # ---- resident loads
            iota_t = consts.tile([P, G, P], bf16)
            nc.sync.dma_start(iota_t[:], iota_d[:].rearrange("p (g q) -> p g q", q=P))
            wa_t = consts.tile([D, D], f32)
            nc.sync.dma_start(wa_t[:], wa_d[:])
            wt_t = consts.tile([D, D], f32)
            nc.sync.dma_start(wt_t[:], wt_d[:])
            w3_t = consts.tile([D, D], bf16)
            nc.sync.dma_start(w3_t[:], w3_d[:])
            b3f_t = consts.tile([P, D], f32)
            nc.sync.dma_start(b3f_t[:], b3f_d[:])

            nc.gpsimd.load_library(library_config.mlp)
            npairs_reg = nc.gpsimd.to_reg(GP)
            qctr = [0]

            class Stream:
                def __init__(self, name, idx_d, seg_d, n_chunks, table_ap, counts,
                             split_head=False):
                    self.name = name
                    self.counts = counts
                    self.off = np.concatenate([[0], np.cumsum(counts)[:-1]]).astype(int)
                    self.table_ap = table_ap
                    self.idx_t = resident.tile([P, n_chunks * 8], i16, tag=f"idx_{name}")
                    nc.scalar.dma_start(self.idx_t[:], idx_d[:])
                    self.seg_t = resident.tile([P, n_chunks], bf16, tag=f"seg_{name}")
                    nc.sync.dma_start(self.seg_t[:], seg_d[:])
                    self.batches = {}
                    self.gather_insts = []

                def batch(self, b):
                    if b not in self.batches:
                        gt = gat.tile([P, G, D], bf16, tag="gat")
                        inst = nc.gpsimd.dma_gather(
                            gt[:],
                            self.table_ap,
                            self.idx_t[:, b * G * 8:(b + 1) * G * 8],
                            GP,
                            npairs_reg,
                            D,
                            queue_num=qctr[0] % 4,
                        )
                        qctr[0] += 1
                        self.gather_insts.append(inst)
                        oh = ohp.tile([P, G, P], bf16, tag="oh")
                        nc.vector.tensor_tensor(
                            out=oh[:],
                            in0=iota_t[:],
                            in1=self.seg_t[:, b * G:(b + 1) * G].broadcast_to([P, G, P]),
                            op=mybir.AluOpType.is_equal,
                        )
                        self.batches[b] = (gt, oh)
                    return self.batches[b]

            lo = Stream("s2lo", s2lo_idx, s2lo_seg, S2LO, x_tab[0:LO, :], C2lo,
                        split_head=True)
            streams2 = [lo]
            if LO < N:
                hi = Stream("s2hi", s2hi_idx, s2hi_seg, S2HI, x_tab[LO:N, :], C2hi,
                            split_head=True)
                streams2.append(hi)

            # ---- stage A: Se[e] = sum_{pairs with edge e} X[v]
            # AllGather fires per piece as its windows finish flushing.
            flushes = [[] for _ in range(NP3)]
            ags = [None] * NP3
            w_ag = [HB[k + 1] // P - 1 for k in range(NP3)]
            for w in range(NW2):
                total_k = sum(int(s.counts[w]) for s in streams2)
                psum_w = winp.tile([P, P], f32, tag="win")
                k = 0
                for s in streams2:
                    for c in range(int(s.off[w]), int(s.off[w]) + int(s.counts[w])):
                        b, cl = divmod(c, G)
                        gt, oh = s.batch(b)
                        nc.tensor.matmul(
                            psum_w[:],
                            lhsT=oh[:, cl, :],
                            rhs=gt[:, cl, :],
                            start=(k == 0),
                            stop=(k == total_k - 1),
                        )
                        k += 1
                st = sep.tile([P, P], bf16, tag="seflush")
                nc.vector.tensor_copy(out=st[:], in_=psum_w[:])
                fl = nc.sync.dma_start(out=se_slice[w * P:(w + 1) * P, :], in_=st[:])
                kp = next(k2 for k2 in range(NP3) if w * P < HB[k2 + 1])
                flushes[kp].append(fl)
                if w == w_ag[kp]:
                    ags[kp] = nc.gpsimd.collective_compute(
                        "AllGather", mybir.AluOpType.bypass,
                        replica_groups=[list(range(ncores))],
                        ins=[se_slice[HB[kp]:HB[kp + 1], :]], outs=[se_p[kp][:]])
                    for f in flushes[kp]:
                        add_dep_helper(ags[kp].ins, f.ins,
                                       reason=f"AG{kp} reads its se_slice piece")

            # ---- stage B: T[v] = sum_{pairs with vertex v} Se[e]
            # one pass per AllGather piece; the dense tail (stages C/D) is
            # interleaved as windows finalize during the last pass.
            streams3 = [Stream(f"s3p{k}", s3_idx_d[k], s3_seg_d[k], S3[k],
                               se_p[k][:], C3[k]) for k in range(NP3)]
            Tt = resident.tile([P, NW3 * P], f32, tag="Tt")
            xd_t = resident.tile([D, VPC], f32, tag="xd")
            nc.scalar.dma_start(xd_t[:], xd_d[:])
            x0h_t = resident.tile([D, VPC], f32, tag="x0h")
            nc.scalar.dma_start(x0h_t[:], x0h_d[:])
            zt_t = resident.tile([D, VPC], bf16, tag="zt")

            RT = 512

            def emit_c_tile(rt):
                s0 = rt * RT
                L = min(RT, VPC - s0)
                pz = zvp.tile([P, RT], f32, tag="zv")
                nc.tensor.matmul(pz[:, :L], lhsT=wa_t[:], rhs=xd_t[:, s0:s0 + L],
                                 start=True, stop=False)
                nc.tensor.matmul(pz[:, :L], lhsT=wt_t[:], rhs=Tt[:, s0:s0 + L],
                                 start=False, stop=True)
                nc.vector.tensor_add(out=zt_t[:, s0:s0 + L], in0=pz[:, :L],
                                     in1=x0h_t[:, s0:s0 + L])
                for ot in range(s0 // P, (s0 + L + P - 1) // P):
                    o0 = ot * P
                    Lo = min(P, VPC - o0)
                    po = outp.tile([P, P], f32, tag="out")
                    nc.tensor.matmul(po[:Lo, :], lhsT=zt_t[:, o0:o0 + Lo], rhs=w3_t[:],
                                     start=True, stop=True)
                    st = sep.tile([P, P], f32, tag="outflush")
                    nc.vector.tensor_tensor(out=st[:Lo, :], in0=po[:Lo, :],
                                            in1=b3f_t[:Lo, :], op=mybir.AluOpType.add)
                    nc.sync.dma_start(out=out_d[o0:o0 + Lo, :], in_=st[:Lo, :])

            n_ctiles = (VPC + RT - 1) // RT
            for kp, s3 in enumerate(streams3):
                last = kp == NP3 - 1
                done_c = 0
                for w in range(NW3):
                    total_k = int(s3.counts[w])
                    if total_k > 0:
                        psum_w = winp.tile([P, P], f32, tag="win")
                        for k, c in enumerate(range(int(s3.off[w]), int(s3.off[w]) + total_k)):
                            b, cl = divmod(c, G)
                            gt, oh = s3.batch(b)
                            nc.tensor.matmul(
                                psum_w[:],
                                lhsT=gt[:, cl, :],
                                rhs=oh[:, cl, :],
                                start=(k == 0),
                                stop=(k == total_k - 1),
                            )
                        if kp == 0:
                            nc.vector.tensor_copy(out=Tt[:, w * P:(w + 1) * P], in_=psum_w[:])
                        else:
                            nc.vector.tensor_add(out=Tt[:, w * P:(w + 1) * P],
                                                 in0=Tt[:, w * P:(w + 1) * P], in1=psum_w[:])
                    # emit any C tiles fully covered by finalized windows
                    if last:
                        while done_c < n_ctiles and (done_c + 1) * RT <= (w + 1) * P:
                            emit_c_tile(done_c)
                            done_c += 1
                if last:
                    while done_c < n_ctiles:
                        emit_c_tile(done_c)
                        done_c += 1

            for kp, s3 in enumerate(streams3):
                for inst in s3.gather_insts:
                    add_dep_helper(inst.ins, ags[kp].ins,
                                   reason=f"pass-{kp} gathers read se_p{kp}")

    return nc


# ---------------------------------------------------------------------------
# entry point
# ---------------------------------------------------------------------------

def _run(inputs, trace=False, M=25000, ncores=8, G=8, lo_split=32768):
    import sys
    if "/opt/trn_rl_repo" not in sys.path:
        sys.path.insert(0, "/opt/trn_rl_repo")
    from concourse.bass_utils import run_bass_kernel_spmd

    in_maps, meta = preprocess(**inputs, M=M, ncores=ncores, G=G, lo_split=lo_split)
    nc = build_program(meta)
    if not nc.is_finalized():
        nc.finalize()
    res = run_bass_kernel_spmd(nc, in_maps, list(range(ncores)), trace=trace)
    vperm = meta["vperm"]
    out = np.concatenate(
        [np.asarray(res.results[i]["out"])[vperm[i]] for i in range(ncores)], axis=0)
    return out, res


def kernel(**inputs):
    out, _ = _run(inputs)
    return out

